# revision 28
# baseline (speedup 1.0000x reference)
"""Trainium2 Bass kernel for nn_AttentionTopologyModule (point-cloud kNN attention).

Contract: kernel(**inputs) takes the FULL unsharded inputs (as produced by
setup_inputs) and returns the FULL [B, C, N] output.  Internally the work is
sharded data-parallel over (batch, query-half): 8 cores, each handling 4096
query points of one batch element (candidates = all 8192 points of that batch
element).  The tiny MLP weights are replicated to every core.

Algorithm per core (all model arithmetic on device):
  setup:  load xyz/feats; sq_m = ||x_m||^2/2; point-projection table
          Qtab[j,:] = [A_j | Bv_j] in DRAM where
          A = feats@W1n.T + xyz@W1x.T  (attn branch, neighbor part)
          Bv = feats@Wvn.T + xyz@Wvx.T (value branch, neighbor part)
  P1 (per 128-query tile, 1-tile software pipeline so the in-order PE/DVE
      queues never stall between tiles):
      nd[q,m] = x_q . x_m - ||x_m||^2/2   (PE matmul, f32; row-equivalent
                ordering to -distance)  -> exact top-16 via DVE
                max8/max_index/match_replace/max8/max_index (the 5 full
                passes are the kernel's irreducible critical path)
      ONE batched indirect DMA gathers the K=16 Qtab rows per query,
      CN[q,:] = [c_q | d_q] per-query offsets via PE matmul (cached in
      SBUF), HV = gathered + CN (DVE, deferred one tile), per-channel
      sum/sumsq of HV accumulated on the PE via per-k ones-matmuls into
      two PSUM banks (one accumulation group per bank).
  AR1:  AllReduce the BN batch stats (training-mode BatchNorm over the whole
        batch spans all cores); fold scale s into w2 / Wo (relu is positively
        homogeneous, gamma=1>0), fold t/s into the cached per-query offsets.
  P2 (A/B software pipeline; gathers pre-issued two tiles ahead):
        re-gather (batched indirect DMA), add CN' (=CN + t/s), relu,
        logits = h.w2' (DVE mult + reduce), softmax over K=16 without
        max-subtraction (logits are O(10)), exp fused with its row-sum on
        ACT, value weighting in place with unnormalized weights, 1/sum
        applied to the reduced row, o = out@(Wo.T*sv)+bo (PE transpose +
        matmul), o stats via ACT accumulators, o stashed to DRAM.
  AR2:  AllReduce o stats; s_o/t_o.
  P3:   BN+relu on stashed o (ACT per-partition affine), residual add
        feats, DMA out [64, 4096].
"""

import sys

import numpy as np

sys.path.insert(0, "/opt/trn_rl_repo")

import concourse.bacc as bacc
import concourse.bass as bass
import concourse.mybir as mybir
import concourse.tile as tile
from concourse.bass import IndirectOffsetOnAxis

F32 = mybir.dt.float32
U32 = mybir.dt.uint32
ALU = mybir.AluOpType
ACTF = mybir.ActivationFunctionType
AX = mybir.AxisListType

C = 64      # channels
K = 16      # neighbors
H = 64      # hidden dim
EPS = 1e-5
NEG = -1.0e30
SQRT_HALF = 0.7071067811865476


def _b(ap, ins_at, count):
    """Insert a broadcast (step 0) dim into an AP at position ins_at."""
    pat = [list(p) for p in ap.ap]
    pat = pat[:ins_at] + [[0, count]] + pat[ins_at:]
    return bass.AP(tensor=ap.tensor, offset=ap.offset, ap=pat)


def build_nc(N=8192, NQ=4096, n_cores=8, tot_pairs=None, tot_pts=None):
    """Build the per-core Bass program (SPMD: same program, per-core inputs)."""
    NT = NQ // 128          # query tiles
    NA = N // 128           # point tiles (tables)
    if tot_pairs is None:
        tot_pairs = n_cores * NQ * K    # elements per channel in BN1/BNv stats
    if tot_pts is None:
        tot_pts = n_cores * NQ          # elements per channel in BNo stats

    nc = bacc.Bacc("TRN2", target_bir_lowering=False, debug=False,
                   num_devices=n_cores)

    xyzc = nc.dram_tensor("xyzc", [3, N], F32, kind="ExternalInput")
    xyzT = nc.dram_tensor("xyzT", [N, 3], F32, kind="ExternalInput")
    fc = nc.dram_tensor("fc", [C, N], F32, kind="ExternalInput")
    RABd = nc.dram_tensor("RAB", [C + 3, 2 * C], F32, kind="ExternalInput")
    RCNd = nc.dram_tensor("RCN", [C + 4, 2 * C], F32, kind="ExternalInput")
    w2d = nc.dram_tensor("w2rep", [128, H], F32, kind="ExternalInput")
    gbd = nc.dram_tensor("gb", [2, 2 * C], F32, kind="ExternalInput")
    ROd = nc.dram_tensor("RO", [C + 1, C], F32, kind="ExternalInput")
    gbod = nc.dram_tensor("gbo", [C, 2], F32, kind="ExternalInput")
    outd = nc.dram_tensor("out", [C, NQ], F32, kind="ExternalOutput")

    with tile.TileContext(nc) as tc:
        import contextlib
        ctx = contextlib.ExitStack()
        with ctx:
            sing = ctx.enter_context(tc.tile_pool(name="sing", bufs=1))
            dram = ctx.enter_context(tc.tile_pool(name="dram", bufs=1, space="DRAM"))
            ndp = ctx.enter_context(tc.tile_pool(name="ndp", bufs=2))
            gp = ctx.enter_context(tc.tile_pool(name="gp", bufs=3))
            scrp = ctx.enter_context(tc.tile_pool(name="scrp", bufs=3))
            lscp = ctx.enter_context(tc.tile_pool(name="lscp", bufs=2))
            cnp_sb = ctx.enter_context(tc.tile_pool(name="cnsb", bufs=1))
            shp = ctx.enter_context(tc.tile_pool(name="shp", bufs=2))
            smp = ctx.enter_context(tc.tile_pool(name="smp", bufs=3))
            otp_sb = ctx.enter_context(tc.tile_pool(name="otsb", bufs=2))
            # PSUM pools (8 banks total): nd 2x2 banks, cn 1, stats 1, tr 1, oT 1
            ndps = ctx.enter_context(tc.tile_pool(name="ndps", bufs=2, space="PSUM"))
            cnps = ctx.enter_context(tc.tile_pool(name="cnps", bufs=1, space="PSUM"))
            stps = ctx.enter_context(tc.tile_pool(name="stps", bufs=1, space="PSUM"))
            st2ps = ctx.enter_context(tc.tile_pool(name="st2ps", bufs=1, space="PSUM"))
            pops = ctx.enter_context(tc.tile_pool(name="pops", bufs=2, space="PSUM"))

            # ---------------- setup ----------------
            # xyz loads first: they gate the sq-row chain and tile 0's nd
            C4 = sing.tile([4, N], F32)
            nc.sync.dma_start(out=C4[0:3, :], in_=xyzc[:, :])
            q4t_0 = sing.tile([4, 128], F32)
            q4t_1 = sing.tile([4, 128], F32)
            nc.vector.memset(q4t_0, -1.0)
            nc.vector.memset(q4t_1, -1.0)
            q4ts = [q4t_0, q4t_1]
            XT = scrp.tile([128, NA, 3], F32, tag="hvn")
            nc.sync.dma_start(out=XT, in_=xyzT[:, :].rearrange("(a p) d -> p a d", p=128))
            RABa = sing.tile([C, 2 * C], F32)
            nc.sync.dma_start(out=RABa, in_=RABd[0:C, :])
            RABx = sing.tile([3, 2 * C], F32)
            nc.sync.dma_start(out=RABx, in_=RABd[C:C + 3, :])
            RCNa = sing.tile([C, 2 * C], F32)
            nc.sync.dma_start(out=RCNa, in_=RCNd[0:C, :])
            RCNx = sing.tile([3, 2 * C], F32)
            nc.sync.dma_start(out=RCNx, in_=RCNd[C:C + 3, :])
            RCNb = sing.tile([1, 2 * C], F32)
            nc.sync.dma_start(out=RCNb, in_=RCNd[C + 3:C + 4, :])
            # negate the xyz rows: c_n = G1c - G1x + b1, d_n = bv - Gvx
            nc.vector.tensor_scalar_mul(RCNx, RCNx, -1.0)
            w2rep = sing.tile([128, H], F32)
            nc.sync.dma_start(out=w2rep, in_=w2d[:, :])
            gRow = sing.tile([1, 2 * C], F32)
            nc.sync.dma_start(out=gRow, in_=gbd[0:1, :])
            bRow = sing.tile([1, 2 * C], F32)
            nc.sync.dma_start(out=bRow, in_=gbd[1:2, :])
            RO = sing.tile([C + 1, C], F32)
            nc.sync.dma_start(out=RO, in_=ROd[:, :])
            gbo = sing.tile([C, 2], F32)
            nc.sync.dma_start(out=gbo, in_=gbod[:, :])
            F_sbq = sing.tile([C, NQ], F32)
            nc.sync.dma_start(out=F_sbq, in_=fc[:, 0:NQ])
            ones1 = sing.tile([1, 128], F32)
            nc.vector.memset(ones1, 1.0)
            ones128 = sing.tile([128, 1], F32)
            nc.vector.memset(ones128, 1.0)
            # identity for PE transpose
            identI = sing.tile([128, 128], mybir.dt.int32)
            nc.gpsimd.iota(identI, pattern=[[1, 128]], base=0, channel_multiplier=-1)
            ident = sing.tile([128, 128], F32)
            nc.vector.tensor_scalar(ident, identI, 0.0, scalar2=None, op0=ALU.is_equal)

            # sq/2 of candidate points -> row 3 of C4
            XTsq = lscp.tile([128, NA * 3], F32, tag="lsc")
            nc.scalar.activation(XTsq, XT.rearrange("p a d -> p (a d)"),
                                 ACTF.Square, scale=SQRT_HALF)
            SQ2 = sing.tile([128, NA], F32)
            nc.vector.tensor_reduce(out=SQ2, in_=XTsq.rearrange("p (a d) -> p a d", d=3),
                                    axis=AX.X, op=ALU.add)
            sqd = dram.tile([128, NA], F32)
            nc.sync.dma_start(out=sqd, in_=SQ2)
            nc.sync.dma_start(out=C4[3:4, :].rearrange("o (a p) -> o a p", p=128),
                              in_=sqd[:, :].rearrange("p a -> a p"))

            idxall = sing.tile([128, NT * K], U32)
            hv_d = dram.tile([NQ, K * 2 * C], F32)
            cnall = sing.tile([128, NT, 2 * C], F32)
            stat_ps = stps.tile([1, 2 * C], F32)    # sum(h|v)
            stat2_ps = st2ps.tile([1, 2 * C], F32)  # sumsq(h|v)
            # accumulated over all tiles and k-slices by per-k PE ones-matmuls
            # (two separate PSUM banks: one accumulation group per bank)

            NB2 = N // 1024   # nd psum tiles per query tile

            # ---------------- P1: kNN + BN stats ----------------
            # 1-tile software pipeline: nd(t+1) is emitted before the stats
            # matmuls of tile t so the in-order PE queue computes the next
            # tile's distances during this tile's topk instead of stalling
            # on the gather->add->square chain that feeds the stats.
            def emit_nd(t):
                qs = slice(t * 128, (t + 1) * 128)
                q4t = q4ts[t % 2]
                nc.scalar.copy(q4t[0:3, :], C4[0:3, qs])
                nd = ndp.tile([128, N], F32)
                for b2 in range(2 * NB2):
                    cs = slice(b2 * 512, (b2 + 1) * 512)
                    ps = ndps.tile([128, 512], F32)
                    nc.tensor.matmul(ps, lhsT=q4t, rhs=C4[:, cs],
                                     start=True, stop=True)
                    nc.scalar.copy(nd[:, cs], ps)
                return nd

            def emit_stats(t, G, sq3):
                for kk in range(K):
                    nc.tensor.matmul(stat_ps, lhsT=ones128,
                                     rhs=G[:, kk, :],
                                     start=(t == 0 and kk == 0),
                                     stop=(t == NT - 1 and kk == K - 1))
                    nc.tensor.matmul(stat2_ps, lhsT=ones128,
                                     rhs=sq3[:, kk, :],
                                     start=(t == 0 and kk == 0),
                                     stop=(t == NT - 1 and kk == K - 1))

            pend_stats = []
            pend_add = None
            nd = emit_nd(0)
            # point projection tables -> DRAM Qtab [N, 128]
            Qtab = dram.tile([N, 2 * C], F32)
            for a in range(NA):
                pt = slice(a * 128, (a + 1) * 128)
                fstr = shp.tile([C, 128], F32, tag="fstr")
                nc.sync.dma_start(out=fstr, in_=fc[:, pt])
                ps = cnps.tile([128, 2 * C], F32, tag="cps")
                nc.tensor.matmul(ps, lhsT=fstr, rhs=RABa,
                                 start=True, stop=False)
                nc.tensor.matmul(ps, lhsT=C4[0:3, pt], rhs=RABx,
                                 start=False, stop=True)
                tsb = cnp_sb.tile([128, 2 * C], F32)
                nc.scalar.copy(tsb, ps)
                nc.sync.dma_start(out=Qtab[pt, :], in_=tsb)

            for t in range(NT):
                qs = slice(t * 128, (t + 1) * 128)
                # exact top-16 (5 passes)
                v8a = smp.tile([128, 8], F32)
                v8b = smp.tile([128, 8], F32)
                nc.vector.max(out=v8a, in_=nd)
                nc.vector.max_index(out=idxall[:, t * K:t * K + 8], in_max=v8a, in_values=nd)
                nc.vector.match_replace(out=nd, in_to_replace=v8a, in_values=nd,
                                        imm_value=NEG)
                nc.vector.max(out=v8b, in_=nd)
                nc.vector.max_index(out=idxall[:, t * K + 8:t * K + 16], in_max=v8b,
                                    in_values=nd)
                # CN = [c_q | d_q] -> persistent SBUF stash
                cps = cnps.tile([128, 2 * C], F32)
                nc.tensor.matmul(cps, lhsT=F_sbq[:, qs], rhs=RCNa, start=True, stop=False)
                nc.tensor.matmul(cps, lhsT=C4[0:3, qs], rhs=RCNx, start=False, stop=False)
                nc.tensor.matmul(cps, lhsT=ones1, rhs=RCNb, start=False, stop=True)
                nc.scalar.copy(cnall[:, t, :], cps)
                # gather the K Qtab rows (one indirect DMA per k: the
                # hardware DGE only supports one offset per partition)
                G = gp.tile([128, K, 2 * C], F32, tag="g")
                for kk in range(K):
                    nc.gpsimd.indirect_dma_start(
                        out=G[:, kk, :], out_offset=None, in_=Qtab[:, :],
                        in_offset=IndirectOffsetOnAxis(
                            ap=idxall[:, t * K + kk:t * K + kk + 1], axis=0))
                # previous tile's CN-add + square (one tile late so the DVE
                # add never waits on its gather between two topk chains)
                if pend_add is not None:
                    tp, Gp = pend_add
                    nc.vector.scalar_tensor_tensor(
                        out=Gp, in0=Gp, scalar=0.0,
                        in1=_b(cnall[:, tp, :], 1, K),
                        op0=ALU.bypass, op1=ALU.add)
                    sqh = scrp.tile([128, K * 2 * C], F32, tag="hvn")
                    nc.scalar.activation(sqh, Gp.rearrange("p k c -> p (k c)"),
                                         ACTF.Square)
                    nc.sync.dma_start(out=hv_d[tp * 128:(tp + 1) * 128, :],
                                      in_=Gp.rearrange("p k c -> p (k c)"))
                    pend_stats.append((tp, Gp,
                                       sqh.rearrange("p (k c) -> p k c", k=K)))
                pend_add = (t, G)
                # next tile's distances (PE) ...
                if t + 1 < NT:
                    nd = emit_nd(t + 1)
                # ... then the pending stats: per-channel sum & sumsq over
                # (q, k) accumulated on the PE across all tiles and k-slices
                if pend_stats:
                    emit_stats(*pend_stats.pop(0))
            tp, Gp = pend_add
            nc.vector.scalar_tensor_tensor(
                out=Gp, in0=Gp, scalar=0.0, in1=_b(cnall[:, tp, :], 1, K),
                op0=ALU.bypass, op1=ALU.add)
            sqh = scrp.tile([128, K * 2 * C], F32, tag="hvn")
            nc.scalar.activation(sqh, Gp.rearrange("p k c -> p (k c)"), ACTF.Square)
            nc.sync.dma_start(out=hv_d[tp * 128:(tp + 1) * 128, :],
                              in_=Gp.rearrange("p k c -> p (k c)"))
            pend_stats.append((tp, Gp, sqh.rearrange("p (k c) -> p k c", k=K)))
            while pend_stats:
                emit_stats(*pend_stats.pop(0))

            def emit_gather(t):
                G2 = gp.tile([128, K, 2 * C], F32, tag="g")
                nc.sync.dma_start(out=G2,
                                  in_=hv_d[t * 128:(t + 1) * 128, :].rearrange(
                                      "p (k c) -> p k c", k=K))
                return G2

            def emit_a(t, G2=None):
                if G2 is None:
                    G2 = emit_gather(t)
                nc.vector.scalar_tensor_tensor(
                    out=G2, in0=G2, scalar=0.0, in1=_b(t128[:, :], 1, K),
                    op0=ALU.bypass, op1=ALU.add)
                HVn = scrp.tile([128, K * 2 * C], F32, tag="hvn")
                nc.scalar.activation(HVn, G2.rearrange("p k c -> p (k c)"), ACTF.Relu)
                HVn3 = HVn.rearrange("p (k c) -> p k c", k=K)
                # logits & softmax over K (no max-subtraction: logits are
                # O(10) and exp is safe in f32)
                lsc = lscp.tile([128, K, H], F32, tag="lsc")
                nc.vector.scalar_tensor_tensor(
                    out=lsc, in0=HVn3[:, :, 0:C], scalar=0.0,
                    in1=_b(w2p[:, :], 1, K), op0=ALU.bypass, op1=ALU.mult)
                logit = smp.tile([128, K], F32)
                nc.vector.tensor_reduce(out=logit, in_=lsc, axis=AX.X, op=ALU.add)
                ex = smp.tile([128, K], F32)
                sume = smp.tile([128, 1], F32)
                nc.scalar.activation(ex, logit, ACTF.Exp, accum_out=sume)
                rec = smp.tile([128, 1], F32)
                nc.vector.reciprocal(rec, sume)
                return HVn3, ex, rec

            def emit_b(t, HVn3, ex, rec):
                qs = slice(t * 128, (t + 1) * 128)
                # weighted sum over K with unnormalized weights (in place
                # over the value half of HVn)
                prod = HVn3[:, :, C:2 * C]
                nc.vector.scalar_tensor_tensor(
                    out=prod, in0=prod, scalar=0.0,
                    in1=_b(ex[:, :], 2, C), op0=ALU.bypass, op1=ALU.mult)
                outq = smp.tile([128, C], F32, tag="outq")
                nc.vector.tensor_reduce(out=outq, in_=prod.rearrange("p k c -> p c k"),
                                        axis=AX.X, op=ALU.add)
                nc.vector.tensor_scalar_mul(outq, outq, rec[:, 0:1])
                # o = (out @ Wo.T * sv) + bo, via transpose + matmul
                tps = pops.tile([C, 128], F32, tag="po")
                nc.tensor.transpose(tps, outq, ident)
                ot5 = ot5s[t % 2]
                nc.scalar.copy(ot5[0:C, :], tps)
                ops_ = pops.tile([C, 128], F32, tag="po")
                nc.tensor.matmul(ops_, lhsT=ROp, rhs=ot5, start=True, stop=True)
                osb = otp_sb.tile([C, 128], F32, tag="osb")
                nc.scalar.activation(osb, ops_, ACTF.Copy,
                                     accum_out=osums[:, t:t + 1])
                nc.sync.dma_start(out=ostash_d[:, qs], in_=osb)
                osq = otp_sb.tile([C, 128], F32, tag="osq")
                nc.scalar.activation(osq, ops_, ACTF.Square,
                                     accum_out=osums2[:, t:t + 1])


            # ---------------- AR1 ----------------
            stats_sb = sing.tile([1, 4 * C], F32)
            nc.vector.tensor_copy(stats_sb[:, 0:2 * C], stat_ps)
            nc.vector.tensor_copy(stats_sb[:, 2 * C:4 * C], stat2_ps)
            bi1 = dram.tile([1, 4 * C], F32)
            bo1 = dram.tile([1, 4 * C], F32)
            nc.sync.dma_start(out=bi1, in_=stats_sb)
            if n_cores > 1:
                nc.gpsimd.collective_compute(
                    "AllReduce", ALU.add,
                    replica_groups=[list(range(n_cores))],
                    ins=[bi1[:, :].opt()], outs=[bo1[:, :].opt()])
            else:
                nc.sync.dma_start(out=bo1[:, :], in_=bi1[:, :])
            stats2 = sing.tile([1, 4 * C], F32)
            nc.sync.dma_start(out=stats2, in_=bo1)

            mean = sing.tile([1, 2 * C], F32)
            nc.vector.tensor_scalar_mul(mean, stats2[:, 0:2 * C], 1.0 / tot_pairs)
            var = sing.tile([1, 2 * C], F32)
            nc.vector.tensor_scalar_mul(var, stats2[:, 2 * C:4 * C], 1.0 / tot_pairs)
            msq = sing.tile([1, 2 * C], F32)
            nc.vector.tensor_mul(msq, mean, mean)
            nc.vector.tensor_sub(var, var, msq)
            nc.vector.tensor_scalar_add(var, var, EPS)
            sdv = sing.tile([1, 2 * C], F32)
            nc.scalar.sqrt(sdv, var)
            rstd = sing.tile([1, 2 * C], F32)
            nc.vector.reciprocal(rstd, sdv)
            svec = sing.tile([1, 2 * C], F32)
            nc.vector.tensor_mul(svec, gRow, rstd)
            tvec = sing.tile([1, 2 * C], F32)
            nc.vector.tensor_mul(tvec, mean, svec)
            nc.vector.tensor_sub(tvec, bRow, tvec)
            sinv = sing.tile([1, 2 * C], F32)
            nc.vector.reciprocal(sinv, svec)
            tps_row = sing.tile([1, 2 * C], F32)   # t/s row for CN'
            nc.vector.tensor_mul(tps_row, tvec, sinv)
            pre_g = [emit_gather(0), emit_gather(1)]
            # replicate s_h across partitions via PE rank-1 broadcast
            # (0-stride partition DMA is not supported by the hardware DGE)
            srep_ps = cnps.tile([128, H], F32, tag="cps")
            nc.tensor.matmul(srep_ps, lhsT=ones1, rhs=svec[:, 0:C],
                             start=True, stop=True)
            srep = sing.tile([128, H], F32)
            nc.scalar.copy(srep, srep_ps)
            sdr = dram.tile([1, 2 * C], F32)
            nc.sync.dma_start(out=sdr, in_=svec)
            sv64 = sing.tile([C, 1], F32)
            nc.sync.dma_start(out=sv64, in_=sdr[0, C:2 * C].rearrange("(p o) -> p o", o=1))
            # fold s into w2 and Wo
            w2p = sing.tile([128, H], F32)
            nc.vector.tensor_mul(w2p, w2rep, srep)
            ROp = sing.tile([C + 1, C], F32)
            nc.vector.tensor_mul(ROp[0:C, :], RO[0:C, :], sv64.to_broadcast([C, C]))
            nc.vector.tensor_copy(ROp[C:C + 1, :], RO[C:C + 1, :])

            t128_ps = cnps.tile([128, 2 * C], F32, tag="cps")
            nc.tensor.matmul(t128_ps, lhsT=ones1, rhs=tps_row, start=True, stop=True)
            t128 = sing.tile([128, 2 * C], F32)
            nc.scalar.copy(t128, t128_ps)
            # (the t/s shift is applied per-tile in P2 on top of the
            # stashed HV = gathered + CN)
            ostash_d = dram.tile([C, NQ], F32)
            osums = sing.tile([C, NT], F32)
            osums2 = sing.tile([C, NT], F32)

            # pre-initialized [.; ones] staging tiles for the output matmul
            ot5_0 = sing.tile([C + 1, 128], F32)
            ot5_1 = sing.tile([C + 1, 128], F32)
            ot5s = [ot5_0, ot5_1]
            nc.vector.memset(ot5s[0][C:C + 1, :], 1.0)
            nc.vector.memset(ot5s[1][C:C + 1, :], 1.0)

            # ---------------- P2: attention + value + output proj ----------------
            # 1-tile software pipeline: stage A(t) = gather + attention front
            # end; stage B(t-1) = value-weighting + output projection.  B is
            # emitted one tile late so the in-order Pool/PE queues never block
            # the next tile's gather on this tile's back end.
            pend = []
            for t in range(NT):
                pend.append((t, emit_a(t, pre_g.pop(0))))
                if t + 2 < NT:
                    pre_g.append(emit_gather(t + 2))
                if len(pend) > 2:
                    tb, ab = pend.pop(0)
                    emit_b(tb, *ab)
            while pend:
                tb, ab = pend.pop(0)
                emit_b(tb, *ab)

            # ---------------- AR2 ----------------
            ost = sing.tile([C, 2], F32)
            nc.vector.tensor_reduce(out=ost[:, 0:1], in_=osums, axis=AX.X, op=ALU.add)
            nc.vector.tensor_reduce(out=ost[:, 1:2], in_=osums2, axis=AX.X, op=ALU.add)
            bi2 = dram.tile([C, 2], F32)
            bo2 = dram.tile([C, 2], F32)
            nc.sync.dma_start(out=bi2, in_=ost)
            if n_cores > 1:
                nc.gpsimd.collective_compute(
                    "AllReduce", ALU.add,
                    replica_groups=[list(range(n_cores))],
                    ins=[bi2[:, :].opt()], outs=[bo2[:, :].opt()])
            else:
                nc.sync.dma_start(out=bo2[:, :], in_=bi2[:, :])
            ost2 = sing.tile([C, 2], F32)
            nc.sync.dma_start(out=ost2, in_=bo2)
            omean = sing.tile([C, 1], F32)
            nc.vector.tensor_scalar_mul(omean, ost2[:, 0:1], 1.0 / tot_pts)
            ovar = sing.tile([C, 1], F32)
            nc.vector.tensor_scalar_mul(ovar, ost2[:, 1:2], 1.0 / tot_pts)
            omsq = sing.tile([C, 1], F32)
            nc.vector.tensor_mul(omsq, omean, omean)
            nc.vector.tensor_sub(ovar, ovar, omsq)
            nc.vector.tensor_scalar_add(ovar, ovar, EPS)
            osd = sing.tile([C, 1], F32)
            nc.scalar.sqrt(osd, ovar)
            orst = sing.tile([C, 1], F32)
            nc.vector.reciprocal(orst, osd)
            so = sing.tile([C, 1], F32)
            nc.vector.tensor_mul(so, gbo[:, 0:1], orst)
            to = sing.tile([C, 1], F32)
            nc.vector.tensor_mul(to, omean, so)
            nc.vector.tensor_sub(to, gbo[:, 1:2], to)

            # ---------------- P3: BN + relu + residual ----------------
            P3CH = min(1024, NQ)
            for j in range(NQ // P3CH):
                js = slice(j * P3CH, (j + 1) * P3CH)
                ob = scrp.tile([C, P3CH], F32, tag="hvn")
                nc.sync.dma_start(out=ob, in_=ostash_d[:, js])
                nc.scalar.activation(ob, ob, ACTF.Relu, bias=to[:, 0:1],
                                     scale=so[:, 0:1])
                nc.vector.scalar_tensor_tensor(out=ob, in0=ob, scalar=0.0,
                                                in1=F_sbq[:, js],
                                                op0=ALU.bypass, op1=ALU.add)
                nc.sync.dma_start(out=outd[:, js], in_=ob)

    nc.compile()
    return nc


def make_in_maps(xyz, feats, W1, b1, g1, be1, W2, b2, Wv, bv, gv, bev,
                 Wo, bo, go, beo, n_cores=8, N=8192, NQ=4096):
    """Shard/lay out the full inputs into per-core input dicts (layout only)."""
    f32 = np.float32
    W1 = np.asarray(W1, f32)
    Wv = np.asarray(Wv, f32)
    # RAB: rows [in-ch(64); xyz(3)], cols [A(64) | Bv(64)]
    RAB = np.concatenate([
        np.concatenate([W1[:, C:2 * C].T, W1[:, 2 * C:2 * C + 3].T], axis=0),
        np.concatenate([Wv[:, 0:C].T, Wv[:, C:C + 3].T], axis=0),
    ], axis=1).astype(f32)
    RCN = np.concatenate([
        np.concatenate([W1[:, 0:C].T, W1[:, 2 * C:2 * C + 3].T,
                        np.asarray(b1, f32)[None, :]], axis=0),
        np.concatenate([np.zeros((C, C), f32), Wv[:, C:C + 3].T,
                        np.asarray(bv, f32)[None, :]], axis=0),
    ], axis=1).astype(f32)
    w2rep = np.ascontiguousarray(np.broadcast_to(np.asarray(W2, f32)[0], (128, H)))
    gbp = np.stack([np.concatenate([np.asarray(g1, f32), np.asarray(gv, f32)]),
                    np.concatenate([np.asarray(be1, f32), np.asarray(bev, f32)])])
    RO = np.concatenate([np.asarray(Wo, f32).T, np.asarray(bo, f32)[None, :]], axis=0)
    gbo = np.stack([np.asarray(go, f32), np.asarray(beo, f32)], axis=1)

    xyz = np.asarray(xyz, f32)
    feats = np.asarray(feats, f32)
    halves = n_cores // xyz.shape[0]      # cores per batch element
    in_maps = []
    for c in range(n_cores):
        b = c // halves
        h = c % halves
        xb = np.roll(xyz[b], -h * NQ, axis=1)
        fb = np.roll(feats[b], -h * NQ, axis=1)
        in_maps.append({
            "xyzc": np.ascontiguousarray(xb),
            "xyzT": np.ascontiguousarray(xb.T),
            "fc": np.ascontiguousarray(fb),
            "RAB": RAB, "RCN": RCN, "w2rep": w2rep, "gb": gbp,
            "RO": np.ascontiguousarray(RO), "gbo": np.ascontiguousarray(gbo),
        })
    return in_maps


_NC_CACHE = {}


def kernel(**inputs):
    from concourse.bass_utils import run_bass_kernel_spmd
    B, _, N = inputs["xyz"].shape
    n_cores = 8
    NQ = N * B // n_cores
    key = (N, NQ, n_cores)
    if key not in _NC_CACHE:
        _NC_CACHE[key] = build_nc(N=N, NQ=NQ, n_cores=n_cores)
    nc = _NC_CACHE[key]
    in_maps = make_in_maps(n_cores=n_cores, N=N, NQ=NQ, **inputs)
    res = run_bass_kernel_spmd(nc, in_maps, core_ids=list(range(n_cores)))
    halves = n_cores // B
    out = np.empty((B, C, N), np.float32)
    for c in range(n_cores):
        b, h = c // halves, c % halves
        out[b][:, h * NQ:(h + 1) * NQ] = res.results[c]["out"]
    return out


# revision 33
# speedup vs baseline: 1.0209x; 1.0209x over previous
"""Trainium2 Bass kernel for nn_AttentionTopologyModule (point-cloud kNN attention).

Contract: kernel(**inputs) takes the FULL unsharded inputs (as produced by
setup_inputs) and returns the FULL [B, C, N] output.  Internally the work is
sharded data-parallel over (batch, query-half): 8 cores, each handling 4096
query points of one batch element (candidates = all 8192 points of that batch
element).  The tiny MLP weights are replicated to every core.

Algorithm per core (all model arithmetic on device):
  setup:  load xyz/feats; sq_m = ||x_m||^2/2; point-projection table
          Qtab[j,:] = [A_j | Bv_j] in DRAM where
          A = feats@W1n.T + xyz@W1x.T  (attn branch, neighbor part)
          Bv = feats@Wvn.T + xyz@Wvx.T (value branch, neighbor part)
  P1 (per 128-query tile, 1-tile software pipeline so the in-order PE/DVE
      queues never stall between tiles):
      nd[q,m] = x_q . x_m - ||x_m||^2/2   (PE matmul, f32; row-equivalent
                ordering to -distance)  -> exact top-16 via DVE
                max8/max_index/match_replace/max8/max_index (the 5 full
                passes are the kernel's irreducible critical path)
      ONE batched indirect DMA gathers the K=16 Qtab rows per query,
      CN[q,:] = [c_q | d_q] per-query offsets via PE matmul (cached in
      SBUF), HV = gathered + CN (DVE, deferred one tile), per-channel
      sum/sumsq of HV accumulated on the PE via per-k ones-matmuls into
      two PSUM banks (one accumulation group per bank).
  AR1:  AllReduce the BN batch stats (training-mode BatchNorm over the whole
        batch spans all cores); fold scale s into w2 / Wo (relu is positively
        homogeneous, gamma=1>0), fold t/s into the cached per-query offsets.
  P2 (A/B software pipeline; gathers pre-issued two tiles ahead):
        re-gather (batched indirect DMA), add CN' (=CN + t/s), relu,
        logits = h.w2' (DVE mult + reduce), softmax over K=16 without
        max-subtraction (logits are O(10)), exp fused with its row-sum on
        ACT, value weighting in place with unnormalized weights, 1/sum
        applied to the reduced row, o = out@(Wo.T*sv)+bo (PE transpose +
        matmul), o stats via ACT accumulators, o stashed to DRAM.
  AR2:  AllReduce o stats; s_o/t_o.
  P3:   BN+relu on stashed o (ACT per-partition affine), residual add
        feats, DMA out [64, 4096].
"""

import sys

import numpy as np

sys.path.insert(0, "/opt/trn_rl_repo")

import concourse.bacc as bacc
import concourse.bass as bass
import concourse.mybir as mybir
import concourse.tile as tile
from concourse.bass import IndirectOffsetOnAxis

F32 = mybir.dt.float32
U32 = mybir.dt.uint32
ALU = mybir.AluOpType
ACTF = mybir.ActivationFunctionType
AX = mybir.AxisListType

C = 64      # channels
K = 16      # neighbors
H = 64      # hidden dim
EPS = 1e-5
NEG = -1.0e30
SQRT_HALF = 0.7071067811865476


def _b(ap, ins_at, count):
    """Insert a broadcast (step 0) dim into an AP at position ins_at."""
    pat = [list(p) for p in ap.ap]
    pat = pat[:ins_at] + [[0, count]] + pat[ins_at:]
    return bass.AP(tensor=ap.tensor, offset=ap.offset, ap=pat)


def build_nc(N=8192, NQ=4096, n_cores=8, tot_pairs=None, tot_pts=None):
    """Build the per-core Bass program (SPMD: same program, per-core inputs)."""
    NT = NQ // 128          # query tiles
    NA = N // 128           # point tiles (tables)
    if tot_pairs is None:
        tot_pairs = n_cores * NQ * K    # elements per channel in BN1/BNv stats
    if tot_pts is None:
        tot_pts = n_cores * NQ          # elements per channel in BNo stats

    nc = bacc.Bacc("TRN2", target_bir_lowering=False, debug=False,
                   num_devices=n_cores)

    xyzc = nc.dram_tensor("xyzc", [3, N], F32, kind="ExternalInput")
    xyzT = nc.dram_tensor("xyzT", [N, 3], F32, kind="ExternalInput")
    fc = nc.dram_tensor("fc", [C, N], F32, kind="ExternalInput")
    RABd = nc.dram_tensor("RAB", [C + 3, 2 * C], F32, kind="ExternalInput")
    RCNd = nc.dram_tensor("RCN", [C + 4, 2 * C], F32, kind="ExternalInput")
    w2d = nc.dram_tensor("w2rep", [128, H], F32, kind="ExternalInput")
    gbd = nc.dram_tensor("gb", [2, 2 * C], F32, kind="ExternalInput")
    ROd = nc.dram_tensor("RO", [C + 1, C], F32, kind="ExternalInput")
    gbod = nc.dram_tensor("gbo", [C, 2], F32, kind="ExternalInput")
    outd = nc.dram_tensor("out", [C, NQ], F32, kind="ExternalOutput")

    with tile.TileContext(nc) as tc:
        import contextlib
        ctx = contextlib.ExitStack()
        with ctx:
            sing = ctx.enter_context(tc.tile_pool(name="sing", bufs=1))
            dram = ctx.enter_context(tc.tile_pool(name="dram", bufs=1, space="DRAM"))
            ndp = ctx.enter_context(tc.tile_pool(name="ndp", bufs=2))
            gp = ctx.enter_context(tc.tile_pool(name="gp", bufs=3))
            scrp = ctx.enter_context(tc.tile_pool(name="scrp", bufs=3))
            lscp = ctx.enter_context(tc.tile_pool(name="lscp", bufs=2))
            cnp_sb = ctx.enter_context(tc.tile_pool(name="cnsb", bufs=1))
            shp = ctx.enter_context(tc.tile_pool(name="shp", bufs=2))
            smp = ctx.enter_context(tc.tile_pool(name="smp", bufs=3))
            otp_sb = ctx.enter_context(tc.tile_pool(name="otsb", bufs=2))
            # PSUM pools (8 banks total): nd 2x2 banks, cn 1, stats 1, tr 1, oT 1
            ndps = ctx.enter_context(tc.tile_pool(name="ndps", bufs=2, space="PSUM"))
            cnps = ctx.enter_context(tc.tile_pool(name="cnps", bufs=1, space="PSUM"))
            stps = ctx.enter_context(tc.tile_pool(name="stps", bufs=1, space="PSUM"))
            st2ps = ctx.enter_context(tc.tile_pool(name="st2ps", bufs=1, space="PSUM"))
            pops = ctx.enter_context(tc.tile_pool(name="pops", bufs=2, space="PSUM"))

            # ---------------- setup ----------------
            # xyz loads first: they gate the sq-row chain and tile 0's nd
            C4 = sing.tile([4, N], F32)
            nc.sync.dma_start(out=C4[0:3, :], in_=xyzc[:, :])
            q4t_0 = sing.tile([4, 128], F32)
            q4t_1 = sing.tile([4, 128], F32)
            nc.vector.memset(q4t_0, -1.0)
            nc.vector.memset(q4t_1, -1.0)
            q4ts = [q4t_0, q4t_1]
            XT = scrp.tile([128, NA, 3], F32, tag="hvn")
            nc.sync.dma_start(out=XT, in_=xyzT[:, :].rearrange("(a p) d -> p a d", p=128))
            RABa = sing.tile([C, 2 * C], F32)
            nc.sync.dma_start(out=RABa, in_=RABd[0:C, :])
            RABx = sing.tile([3, 2 * C], F32)
            nc.sync.dma_start(out=RABx, in_=RABd[C:C + 3, :])
            RCNa = sing.tile([C, 2 * C], F32)
            nc.sync.dma_start(out=RCNa, in_=RCNd[0:C, :])
            RCNx = sing.tile([3, 2 * C], F32)
            nc.sync.dma_start(out=RCNx, in_=RCNd[C:C + 3, :])
            RCNb = sing.tile([1, 2 * C], F32)
            nc.sync.dma_start(out=RCNb, in_=RCNd[C + 3:C + 4, :])
            # negate the xyz rows: c_n = G1c - G1x + b1, d_n = bv - Gvx
            nc.vector.tensor_scalar_mul(RCNx, RCNx, -1.0)
            w2rep = sing.tile([128, H], F32)
            nc.sync.dma_start(out=w2rep, in_=w2d[:, :])
            gRow = sing.tile([1, 2 * C], F32)
            nc.sync.dma_start(out=gRow, in_=gbd[0:1, :])
            bRow = sing.tile([1, 2 * C], F32)
            nc.sync.dma_start(out=bRow, in_=gbd[1:2, :])
            RO = sing.tile([C + 1, C], F32)
            nc.sync.dma_start(out=RO, in_=ROd[:, :])
            gbo = sing.tile([C, 2], F32)
            nc.sync.dma_start(out=gbo, in_=gbod[:, :])
            F_sbq = sing.tile([C, NQ], F32)
            nc.sync.dma_start(out=F_sbq, in_=fc[:, 0:NQ])
            ones1 = sing.tile([1, 128], F32)
            nc.vector.memset(ones1, 1.0)
            ones128 = sing.tile([128, 1], F32)
            nc.vector.memset(ones128, 1.0)
            # identity for PE transpose
            identI = sing.tile([128, 128], mybir.dt.int32)
            nc.gpsimd.iota(identI, pattern=[[1, 128]], base=0, channel_multiplier=-1)
            ident = sing.tile([128, 128], F32)
            nc.vector.tensor_scalar(ident, identI, 0.0, scalar2=None, op0=ALU.is_equal)

            # sq/2 of candidate points -> row 3 of C4
            XTsq = lscp.tile([128, NA * 3], F32, tag="lsc")
            nc.scalar.activation(XTsq, XT.rearrange("p a d -> p (a d)"),
                                 ACTF.Square, scale=SQRT_HALF)
            SQ2 = sing.tile([128, NA], F32)
            nc.vector.tensor_reduce(out=SQ2, in_=XTsq.rearrange("p (a d) -> p a d", d=3),
                                    axis=AX.X, op=ALU.add)
            sqd = dram.tile([128, NA], F32)
            nc.sync.dma_start(out=sqd, in_=SQ2)
            nc.sync.dma_start(out=C4[3:4, :].rearrange("o (a p) -> o a p", p=128),
                              in_=sqd[:, :].rearrange("p a -> a p"))

            idxall = sing.tile([128, NT * K], U32)
            hv_d = dram.tile([NQ, K * 2 * C], F32)
            cnall = sing.tile([128, NT, 2 * C], F32)
            stat_ps = stps.tile([1, 2 * C], F32)    # sum(h|v)
            stat2_ps = st2ps.tile([1, 2 * C], F32)  # sumsq(h|v)
            # accumulated over all tiles and k-slices by per-k PE ones-matmuls
            # (two separate PSUM banks: one accumulation group per bank)

            NB2 = N // 1024   # nd psum tiles per query tile

            # ---------------- P1: kNN + BN stats ----------------
            # 1-tile software pipeline: nd(t+1) is emitted before the stats
            # matmuls of tile t so the in-order PE queue computes the next
            # tile's distances during this tile's topk instead of stalling
            # on the gather->add->square chain that feeds the stats.
            def emit_nd(t):
                qs = slice(t * 128, (t + 1) * 128)
                q4t = q4ts[t % 2]
                nc.scalar.copy(q4t[0:3, :], C4[0:3, qs])
                nd = ndp.tile([128, N], F32)
                for b2 in range(2 * NB2):
                    cs = slice(b2 * 512, (b2 + 1) * 512)
                    ps = ndps.tile([128, 512], F32)
                    nc.tensor.matmul(ps, lhsT=q4t, rhs=C4[:, cs],
                                     start=True, stop=True)
                    nc.scalar.copy(nd[:, cs], ps)
                return nd

            def emit_stats(t, G, sq3):
                for kk in range(K):
                    nc.tensor.matmul(stat_ps, lhsT=ones128,
                                     rhs=G[:, kk, :],
                                     start=(t == 0 and kk == 0),
                                     stop=(t == NT - 1 and kk == K - 1))
                    nc.tensor.matmul(stat2_ps, lhsT=ones128,
                                     rhs=sq3[:, kk, :],
                                     start=(t == 0 and kk == 0),
                                     stop=(t == NT - 1 and kk == K - 1))

            pend_stats = []
            pend_add = None
            nd = emit_nd(0)
            # point projection tables -> DRAM Qtab [N, 128]
            Qtab = dram.tile([N, 2 * C], F32)
            for a in range(NA):
                pt = slice(a * 128, (a + 1) * 128)
                fstr = shp.tile([C, 128], F32, tag="fstr")
                nc.sync.dma_start(out=fstr, in_=fc[:, pt])
                ps = cnps.tile([128, 2 * C], F32, tag="cps")
                nc.tensor.matmul(ps, lhsT=fstr, rhs=RABa,
                                 start=True, stop=False)
                nc.tensor.matmul(ps, lhsT=C4[0:3, pt], rhs=RABx,
                                 start=False, stop=True)
                tsb = cnp_sb.tile([128, 2 * C], F32)
                nc.scalar.copy(tsb, ps)
                nc.sync.dma_start(out=Qtab[pt, :], in_=tsb)

            for t in range(NT):
                qs = slice(t * 128, (t + 1) * 128)
                # exact top-16 (5 passes)
                v8a = smp.tile([128, 8], F32)
                v8b = smp.tile([128, 8], F32)
                nc.vector.max(out=v8a, in_=nd)
                nc.vector.max_index(out=idxall[:, t * K:t * K + 8], in_max=v8a, in_values=nd)
                # first half of the gathers can start as soon as the first
                # 8 indices are known, overlapping the rest of the topk
                G = gp.tile([128, K, 2 * C], F32, tag="g")
                for kk in range(8):
                    nc.gpsimd.indirect_dma_start(
                        out=G[:, kk, :], out_offset=None, in_=Qtab[:, :],
                        in_offset=IndirectOffsetOnAxis(
                            ap=idxall[:, t * K + kk:t * K + kk + 1], axis=0))
                nc.vector.match_replace(out=nd, in_to_replace=v8a, in_values=nd,
                                        imm_value=NEG)
                nc.vector.max(out=v8b, in_=nd)
                nc.vector.max_index(out=idxall[:, t * K + 8:t * K + 16], in_max=v8b,
                                    in_values=nd)
                # CN = [c_q | d_q] -> persistent SBUF stash
                cps = cnps.tile([128, 2 * C], F32)
                nc.tensor.matmul(cps, lhsT=F_sbq[:, qs], rhs=RCNa, start=True, stop=False)
                nc.tensor.matmul(cps, lhsT=C4[0:3, qs], rhs=RCNx, start=False, stop=False)
                nc.tensor.matmul(cps, lhsT=ones1, rhs=RCNb, start=False, stop=True)
                nc.scalar.copy(cnall[:, t, :], cps)
                # second half of the gathers (one indirect DMA per k: the
                # hardware DGE only supports one offset per partition)
                for kk in range(8, K):
                    nc.gpsimd.indirect_dma_start(
                        out=G[:, kk, :], out_offset=None, in_=Qtab[:, :],
                        in_offset=IndirectOffsetOnAxis(
                            ap=idxall[:, t * K + kk:t * K + kk + 1], axis=0))
                # previous tile's CN-add + square (one tile late so the DVE
                # add never waits on its gather between two topk chains)
                if pend_add is not None:
                    tp, Gp = pend_add
                    nc.vector.scalar_tensor_tensor(
                        out=Gp, in0=Gp, scalar=0.0,
                        in1=_b(cnall[:, tp, :], 1, K),
                        op0=ALU.bypass, op1=ALU.add)
                    sqh = scrp.tile([128, K * 2 * C], F32, tag="hvn")
                    nc.scalar.activation(sqh, Gp.rearrange("p k c -> p (k c)"),
                                         ACTF.Square)
                    nc.sync.dma_start(out=hv_d[tp * 128:(tp + 1) * 128, :],
                                      in_=Gp.rearrange("p k c -> p (k c)"))
                    pend_stats.append((tp, Gp,
                                       sqh.rearrange("p (k c) -> p k c", k=K)))
                pend_add = (t, G)
                # next tile's distances (PE) ...
                if t + 1 < NT:
                    nd = emit_nd(t + 1)
                # ... then the pending stats: per-channel sum & sumsq over
                # (q, k) accumulated on the PE across all tiles and k-slices
                if pend_stats:
                    emit_stats(*pend_stats.pop(0))
            tp, Gp = pend_add
            nc.vector.scalar_tensor_tensor(
                out=Gp, in0=Gp, scalar=0.0, in1=_b(cnall[:, tp, :], 1, K),
                op0=ALU.bypass, op1=ALU.add)
            sqh = scrp.tile([128, K * 2 * C], F32, tag="hvn")
            nc.scalar.activation(sqh, Gp.rearrange("p k c -> p (k c)"), ACTF.Square)
            nc.sync.dma_start(out=hv_d[tp * 128:(tp + 1) * 128, :],
                              in_=Gp.rearrange("p k c -> p (k c)"))
            pend_stats.append((tp, Gp, sqh.rearrange("p (k c) -> p k c", k=K)))
            while pend_stats:
                emit_stats(*pend_stats.pop(0))

            def emit_gather(t):
                # broadcast-fill with the t/s shift on ACT, then the stashed
                # HV accumulates on top via a SWDGE CCE-add DMA (keeps the
                # per-tile broadcast add off the DVE critical path)
                G2 = gp.tile([128, K, 2 * C], F32, tag="g")
                nc.scalar.activation(G2, _b(t128[:, :], 1, K), ACTF.Copy)
                nc.gpsimd.dma_start(
                    out=G2.rearrange("p k c -> p (k c)"),
                    in_=hv_d[t * 128:(t + 1) * 128, :], accum_op=ALU.add)
                return G2

            def emit_a(t, G2=None):
                if G2 is None:
                    G2 = emit_gather(t)
                HVn = scrp.tile([128, K * 2 * C], F32, tag="hvn")
                nc.scalar.activation(HVn, G2.rearrange("p k c -> p (k c)"), ACTF.Relu)
                HVn3 = HVn.rearrange("p (k c) -> p k c", k=K)
                # logits & softmax over K (no max-subtraction: logits are
                # O(10) and exp is safe in f32)
                lsc = lscp.tile([128, K, H], F32, tag="lsc")
                nc.vector.scalar_tensor_tensor(
                    out=lsc, in0=HVn3[:, :, 0:C], scalar=0.0,
                    in1=_b(w2p[:, :], 1, K), op0=ALU.bypass, op1=ALU.mult)
                logit = smp.tile([128, K], F32)
                nc.vector.tensor_reduce(out=logit, in_=lsc, axis=AX.X, op=ALU.add)
                ex = smp.tile([128, K], F32)
                sume = smp.tile([128, 1], F32)
                nc.scalar.activation(ex, logit, ACTF.Exp, accum_out=sume)
                rec = smp.tile([128, 1], F32)
                nc.vector.reciprocal(rec, sume)
                return HVn3, ex, rec

            def emit_b(t, HVn3, ex, rec):
                qs = slice(t * 128, (t + 1) * 128)
                # weighted sum over K with unnormalized weights (in place
                # over the value half of HVn)
                prod = HVn3[:, :, C:2 * C]
                nc.vector.scalar_tensor_tensor(
                    out=prod, in0=prod, scalar=0.0,
                    in1=_b(ex[:, :], 2, C), op0=ALU.bypass, op1=ALU.mult)
                outq = smp.tile([128, C], F32, tag="outq")
                nc.vector.tensor_reduce(out=outq, in_=prod.rearrange("p k c -> p c k"),
                                        axis=AX.X, op=ALU.add)
                nc.vector.tensor_scalar_mul(outq, outq, rec[:, 0:1])
                # o = (out @ Wo.T * sv) + bo, via transpose + matmul
                tps = pops.tile([C, 128], F32, tag="po")
                nc.tensor.transpose(tps, outq, ident)
                ot5 = ot5s[t % 2]
                nc.scalar.copy(ot5[0:C, :], tps)
                ops_ = pops.tile([C, 128], F32, tag="po")
                nc.tensor.matmul(ops_, lhsT=ROp, rhs=ot5, start=True, stop=True)
                osb = otp_sb.tile([C, 128], F32, tag="osb")
                nc.scalar.activation(osb, ops_, ACTF.Copy,
                                     accum_out=osums[:, t:t + 1])
                nc.sync.dma_start(out=ostash_d[:, qs], in_=osb)
                osq = otp_sb.tile([C, 128], F32, tag="osq")
                nc.scalar.activation(osq, ops_, ACTF.Square,
                                     accum_out=osums2[:, t:t + 1])


            # ---------------- AR1 ----------------
            stats_sb = sing.tile([1, 4 * C], F32)
            nc.vector.tensor_copy(stats_sb[:, 0:2 * C], stat_ps)
            nc.vector.tensor_copy(stats_sb[:, 2 * C:4 * C], stat2_ps)
            bi1 = dram.tile([1, 4 * C], F32)
            bo1 = dram.tile([1, 4 * C], F32)
            nc.sync.dma_start(out=bi1, in_=stats_sb)
            if n_cores > 1:
                nc.gpsimd.collective_compute(
                    "AllReduce", ALU.add,
                    replica_groups=[list(range(n_cores))],
                    ins=[bi1[:, :].opt()], outs=[bo1[:, :].opt()])
            else:
                nc.sync.dma_start(out=bo1[:, :], in_=bi1[:, :])
            stats2 = sing.tile([1, 4 * C], F32)
            nc.sync.dma_start(out=stats2, in_=bo1)

            mean = sing.tile([1, 2 * C], F32)
            nc.vector.tensor_scalar_mul(mean, stats2[:, 0:2 * C], 1.0 / tot_pairs)
            var = sing.tile([1, 2 * C], F32)
            nc.vector.tensor_scalar_mul(var, stats2[:, 2 * C:4 * C], 1.0 / tot_pairs)
            msq = sing.tile([1, 2 * C], F32)
            nc.vector.tensor_mul(msq, mean, mean)
            nc.vector.tensor_sub(var, var, msq)
            nc.vector.tensor_scalar_add(var, var, EPS)
            sdv = sing.tile([1, 2 * C], F32)
            nc.scalar.sqrt(sdv, var)
            rstd = sing.tile([1, 2 * C], F32)
            nc.vector.reciprocal(rstd, sdv)
            svec = sing.tile([1, 2 * C], F32)
            nc.vector.tensor_mul(svec, gRow, rstd)
            tvec = sing.tile([1, 2 * C], F32)
            nc.vector.tensor_mul(tvec, mean, svec)
            nc.vector.tensor_sub(tvec, bRow, tvec)
            sinv = sing.tile([1, 2 * C], F32)
            nc.vector.reciprocal(sinv, svec)
            tps_row = sing.tile([1, 2 * C], F32)   # t/s row for CN'
            nc.vector.tensor_mul(tps_row, tvec, sinv)
            # replicate s_h across partitions via PE rank-1 broadcast
            # (0-stride partition DMA is not supported by the hardware DGE)
            srep_ps = cnps.tile([128, H], F32, tag="cps")
            nc.tensor.matmul(srep_ps, lhsT=ones1, rhs=svec[:, 0:C],
                             start=True, stop=True)
            srep = sing.tile([128, H], F32)
            nc.scalar.copy(srep, srep_ps)
            sdr = dram.tile([1, 2 * C], F32)
            nc.sync.dma_start(out=sdr, in_=svec)
            sv64 = sing.tile([C, 1], F32)
            nc.sync.dma_start(out=sv64, in_=sdr[0, C:2 * C].rearrange("(p o) -> p o", o=1))
            # fold s into w2 and Wo
            w2p = sing.tile([128, H], F32)
            nc.vector.tensor_mul(w2p, w2rep, srep)
            ROp = sing.tile([C + 1, C], F32)
            nc.vector.tensor_mul(ROp[0:C, :], RO[0:C, :], sv64.to_broadcast([C, C]))
            nc.vector.tensor_copy(ROp[C:C + 1, :], RO[C:C + 1, :])

            t128_ps = cnps.tile([128, 2 * C], F32, tag="cps")
            nc.tensor.matmul(t128_ps, lhsT=ones1, rhs=tps_row, start=True, stop=True)
            t128 = sing.tile([128, 2 * C], F32)
            nc.scalar.copy(t128, t128_ps)
            pre_g = [emit_gather(0), emit_gather(1)]
            # (the t/s shift is applied per-tile in P2 on top of the
            # stashed HV = gathered + CN)
            ostash_d = dram.tile([C, NQ], F32)
            osums = sing.tile([C, NT], F32)
            osums2 = sing.tile([C, NT], F32)

            # pre-initialized [.; ones] staging tiles for the output matmul
            ot5_0 = sing.tile([C + 1, 128], F32)
            ot5_1 = sing.tile([C + 1, 128], F32)
            ot5s = [ot5_0, ot5_1]
            nc.vector.memset(ot5s[0][C:C + 1, :], 1.0)
            nc.vector.memset(ot5s[1][C:C + 1, :], 1.0)

            # ---------------- P2: attention + value + output proj ----------------
            # 1-tile software pipeline: stage A(t) = gather + attention front
            # end; stage B(t-1) = value-weighting + output projection.  B is
            # emitted one tile late so the in-order Pool/PE queues never block
            # the next tile's gather on this tile's back end.
            pend = []
            for t in range(NT):
                pend.append((t, emit_a(t, pre_g.pop(0))))
                if t + 2 < NT:
                    pre_g.append(emit_gather(t + 2))
                if len(pend) > 2:
                    tb, ab = pend.pop(0)
                    emit_b(tb, *ab)
            while pend:
                tb, ab = pend.pop(0)
                emit_b(tb, *ab)

            # ---------------- AR2 ----------------
            ost = sing.tile([C, 2], F32)
            nc.vector.tensor_reduce(out=ost[:, 0:1], in_=osums, axis=AX.X, op=ALU.add)
            nc.vector.tensor_reduce(out=ost[:, 1:2], in_=osums2, axis=AX.X, op=ALU.add)
            bi2 = dram.tile([C, 2], F32)
            bo2 = dram.tile([C, 2], F32)
            nc.sync.dma_start(out=bi2, in_=ost)
            if n_cores > 1:
                nc.gpsimd.collective_compute(
                    "AllReduce", ALU.add,
                    replica_groups=[list(range(n_cores))],
                    ins=[bi2[:, :].opt()], outs=[bo2[:, :].opt()])
            else:
                nc.sync.dma_start(out=bo2[:, :], in_=bi2[:, :])
            ost2 = sing.tile([C, 2], F32)
            nc.sync.dma_start(out=ost2, in_=bo2)
            omean = sing.tile([C, 1], F32)
            nc.vector.tensor_scalar_mul(omean, ost2[:, 0:1], 1.0 / tot_pts)
            ovar = sing.tile([C, 1], F32)
            nc.vector.tensor_scalar_mul(ovar, ost2[:, 1:2], 1.0 / tot_pts)
            omsq = sing.tile([C, 1], F32)
            nc.vector.tensor_mul(omsq, omean, omean)
            nc.vector.tensor_sub(ovar, ovar, omsq)
            nc.vector.tensor_scalar_add(ovar, ovar, EPS)
            osd = sing.tile([C, 1], F32)
            nc.scalar.sqrt(osd, ovar)
            orst = sing.tile([C, 1], F32)
            nc.vector.reciprocal(orst, osd)
            so = sing.tile([C, 1], F32)
            nc.vector.tensor_mul(so, gbo[:, 0:1], orst)
            to = sing.tile([C, 1], F32)
            nc.vector.tensor_mul(to, omean, so)
            nc.vector.tensor_sub(to, gbo[:, 1:2], to)

            # ---------------- P3: BN + relu + residual ----------------
            P3CH = min(1024, NQ)
            for j in range(NQ // P3CH):
                js = slice(j * P3CH, (j + 1) * P3CH)
                ob = scrp.tile([C, P3CH], F32, tag="hvn")
                nc.sync.dma_start(out=ob, in_=ostash_d[:, js])
                nc.scalar.activation(ob, ob, ACTF.Relu, bias=to[:, 0:1],
                                     scale=so[:, 0:1])
                nc.vector.scalar_tensor_tensor(out=ob, in0=ob, scalar=0.0,
                                                in1=F_sbq[:, js],
                                                op0=ALU.bypass, op1=ALU.add)
                nc.sync.dma_start(out=outd[:, js], in_=ob)

    nc.compile()
    return nc


def make_in_maps(xyz, feats, W1, b1, g1, be1, W2, b2, Wv, bv, gv, bev,
                 Wo, bo, go, beo, n_cores=8, N=8192, NQ=4096):
    """Shard/lay out the full inputs into per-core input dicts (layout only)."""
    f32 = np.float32
    W1 = np.asarray(W1, f32)
    Wv = np.asarray(Wv, f32)
    # RAB: rows [in-ch(64); xyz(3)], cols [A(64) | Bv(64)]
    RAB = np.concatenate([
        np.concatenate([W1[:, C:2 * C].T, W1[:, 2 * C:2 * C + 3].T], axis=0),
        np.concatenate([Wv[:, 0:C].T, Wv[:, C:C + 3].T], axis=0),
    ], axis=1).astype(f32)
    RCN = np.concatenate([
        np.concatenate([W1[:, 0:C].T, W1[:, 2 * C:2 * C + 3].T,
                        np.asarray(b1, f32)[None, :]], axis=0),
        np.concatenate([np.zeros((C, C), f32), Wv[:, C:C + 3].T,
                        np.asarray(bv, f32)[None, :]], axis=0),
    ], axis=1).astype(f32)
    w2rep = np.ascontiguousarray(np.broadcast_to(np.asarray(W2, f32)[0], (128, H)))
    gbp = np.stack([np.concatenate([np.asarray(g1, f32), np.asarray(gv, f32)]),
                    np.concatenate([np.asarray(be1, f32), np.asarray(bev, f32)])])
    RO = np.concatenate([np.asarray(Wo, f32).T, np.asarray(bo, f32)[None, :]], axis=0)
    gbo = np.stack([np.asarray(go, f32), np.asarray(beo, f32)], axis=1)

    xyz = np.asarray(xyz, f32)
    feats = np.asarray(feats, f32)
    halves = n_cores // xyz.shape[0]      # cores per batch element
    in_maps = []
    for c in range(n_cores):
        b = c // halves
        h = c % halves
        xb = np.roll(xyz[b], -h * NQ, axis=1)
        fb = np.roll(feats[b], -h * NQ, axis=1)
        in_maps.append({
            "xyzc": np.ascontiguousarray(xb),
            "xyzT": np.ascontiguousarray(xb.T),
            "fc": np.ascontiguousarray(fb),
            "RAB": RAB, "RCN": RCN, "w2rep": w2rep, "gb": gbp,
            "RO": np.ascontiguousarray(RO), "gbo": np.ascontiguousarray(gbo),
        })
    return in_maps


_NC_CACHE = {}


def kernel(**inputs):
    from concourse.bass_utils import run_bass_kernel_spmd
    B, _, N = inputs["xyz"].shape
    n_cores = 8
    NQ = N * B // n_cores
    key = (N, NQ, n_cores)
    if key not in _NC_CACHE:
        _NC_CACHE[key] = build_nc(N=N, NQ=NQ, n_cores=n_cores)
    nc = _NC_CACHE[key]
    in_maps = make_in_maps(n_cores=n_cores, N=N, NQ=NQ, **inputs)
    res = run_bass_kernel_spmd(nc, in_maps, core_ids=list(range(n_cores)))
    halves = n_cores // B
    out = np.empty((B, C, N), np.float32)
    for c in range(n_cores):
        b, h = c // halves, c % halves
        out[b][:, h * NQ:(h + 1) * NQ] = res.results[c]["out"]
    return out


# revision 35
# speedup vs baseline: 1.0384x; 1.0171x over previous
"""Trainium2 Bass kernel for nn_AttentionTopologyModule (point-cloud kNN attention).

Contract: kernel(**inputs) takes the FULL unsharded inputs (as produced by
setup_inputs) and returns the FULL [B, C, N] output.  Internally the work is
sharded data-parallel over (batch, query-half): 8 cores, each handling 4096
query points of one batch element (candidates = all 8192 points of that batch
element).  The tiny MLP weights are replicated to every core.

Algorithm per core (all model arithmetic on device):
  setup:  load xyz/feats; sq_m = ||x_m||^2/2; point-projection table
          Qtab[j,:] = [A_j | Bv_j] in DRAM where
          A = feats@W1n.T + xyz@W1x.T  (attn branch, neighbor part)
          Bv = feats@Wvn.T + xyz@Wvx.T (value branch, neighbor part)
  P1 (per 128-query tile, 1-tile software pipeline so the in-order PE/DVE
      queues never stall between tiles):
      nd[q,m] = x_q . x_m - ||x_m||^2/2   (PE matmul, f32; row-equivalent
                ordering to -distance)  -> exact top-16 via DVE
                max8/max_index/match_replace/max8/max_index (the 5 full
                passes are the kernel's irreducible critical path)
      ONE batched indirect DMA gathers the K=16 Qtab rows per query,
      CN[q,:] = [c_q | d_q] per-query offsets via PE matmul (cached in
      SBUF), HV = gathered + CN (DVE, deferred one tile), per-channel
      sum/sumsq of HV accumulated on the PE via per-k ones-matmuls into
      two PSUM banks (one accumulation group per bank).
  AR1:  AllReduce the BN batch stats (training-mode BatchNorm over the whole
        batch spans all cores); fold scale s into w2 / Wo (relu is positively
        homogeneous, gamma=1>0), fold t/s into the cached per-query offsets.
  P2 (A/B software pipeline; gathers pre-issued two tiles ahead):
        re-gather (batched indirect DMA), add CN' (=CN + t/s), relu,
        logits = h.w2' (DVE mult + reduce), softmax over K=16 without
        max-subtraction (logits are O(10)), exp fused with its row-sum on
        ACT, value weighting in place with unnormalized weights, 1/sum
        applied to the reduced row, o = out@(Wo.T*sv)+bo (PE transpose +
        matmul), o stats via ACT accumulators, o stashed to DRAM.
  AR2:  AllReduce o stats; s_o/t_o.
  P3:   BN+relu on stashed o (ACT per-partition affine), residual add
        feats, DMA out [64, 4096].
"""

import sys

import numpy as np

sys.path.insert(0, "/opt/trn_rl_repo")

import concourse.bacc as bacc
import concourse.bass as bass
import concourse.mybir as mybir
import concourse.tile as tile
from concourse.bass import IndirectOffsetOnAxis

F32 = mybir.dt.float32
U32 = mybir.dt.uint32
ALU = mybir.AluOpType
ACTF = mybir.ActivationFunctionType
AX = mybir.AxisListType

C = 64      # channels
K = 16      # neighbors
H = 64      # hidden dim
EPS = 1e-5
NEG = -1.0e30
SQRT_HALF = 0.7071067811865476


def _b(ap, ins_at, count):
    """Insert a broadcast (step 0) dim into an AP at position ins_at."""
    pat = [list(p) for p in ap.ap]
    pat = pat[:ins_at] + [[0, count]] + pat[ins_at:]
    return bass.AP(tensor=ap.tensor, offset=ap.offset, ap=pat)


def build_nc(N=8192, NQ=4096, n_cores=8, tot_pairs=None, tot_pts=None):
    """Build the per-core Bass program (SPMD: same program, per-core inputs)."""
    NT = NQ // 128          # query tiles
    NA = N // 128           # point tiles (tables)
    if tot_pairs is None:
        tot_pairs = n_cores * NQ * K    # elements per channel in BN1/BNv stats
    if tot_pts is None:
        tot_pts = n_cores * NQ          # elements per channel in BNo stats

    nc = bacc.Bacc("TRN2", target_bir_lowering=False, debug=False,
                   num_devices=n_cores)

    xyzc = nc.dram_tensor("xyzc", [3, N], F32, kind="ExternalInput")
    xyzT = nc.dram_tensor("xyzT", [N, 3], F32, kind="ExternalInput")
    fc = nc.dram_tensor("fc", [C, N], F32, kind="ExternalInput")
    RABd = nc.dram_tensor("RAB", [C + 3, 2 * C], F32, kind="ExternalInput")
    RCNd = nc.dram_tensor("RCN", [C + 4, 2 * C], F32, kind="ExternalInput")
    w2d = nc.dram_tensor("w2rep", [128, H], F32, kind="ExternalInput")
    gbd = nc.dram_tensor("gb", [2, 2 * C], F32, kind="ExternalInput")
    ROd = nc.dram_tensor("RO", [C + 1, C], F32, kind="ExternalInput")
    gbod = nc.dram_tensor("gbo", [C, 2], F32, kind="ExternalInput")
    outd = nc.dram_tensor("out", [C, NQ], F32, kind="ExternalOutput")

    with tile.TileContext(nc) as tc:
        import contextlib
        ctx = contextlib.ExitStack()
        with ctx:
            sing = ctx.enter_context(tc.tile_pool(name="sing", bufs=1))
            dram = ctx.enter_context(tc.tile_pool(name="dram", bufs=1, space="DRAM"))
            ndp = ctx.enter_context(tc.tile_pool(name="ndp", bufs=2))
            gp = ctx.enter_context(tc.tile_pool(name="gp", bufs=3))
            scrp = ctx.enter_context(tc.tile_pool(name="scrp", bufs=3))
            lscp = ctx.enter_context(tc.tile_pool(name="lscp", bufs=2))
            cnp_sb = ctx.enter_context(tc.tile_pool(name="cnsb", bufs=1))
            shp = ctx.enter_context(tc.tile_pool(name="shp", bufs=2))
            smp = ctx.enter_context(tc.tile_pool(name="smp", bufs=3))
            otp_sb = ctx.enter_context(tc.tile_pool(name="otsb", bufs=2))
            # PSUM pools (8 banks total): nd 2x2 banks, cn 1, stats 1, tr 1, oT 1
            ndps = ctx.enter_context(tc.tile_pool(name="ndps", bufs=2, space="PSUM"))
            cnps = ctx.enter_context(tc.tile_pool(name="cnps", bufs=1, space="PSUM"))
            stps = ctx.enter_context(tc.tile_pool(name="stps", bufs=1, space="PSUM"))
            st2ps = ctx.enter_context(tc.tile_pool(name="st2ps", bufs=1, space="PSUM"))
            pops = ctx.enter_context(tc.tile_pool(name="pops", bufs=2, space="PSUM"))

            # ---------------- setup ----------------
            # xyz loads first: they gate the sq-row chain and tile 0's nd
            C4 = sing.tile([4, N], F32)
            nc.sync.dma_start(out=C4[0:3, :], in_=xyzc[:, :])
            q4t_0 = sing.tile([4, 128], F32)
            q4t_1 = sing.tile([4, 128], F32)
            nc.vector.memset(q4t_0, -1.0)
            nc.vector.memset(q4t_1, -1.0)
            q4ts = [q4t_0, q4t_1]
            XT = scrp.tile([128, NA, 3], F32, tag="hvn")
            nc.sync.dma_start(out=XT, in_=xyzT[:, :].rearrange("(a p) d -> p a d", p=128))
            RABa = sing.tile([C, 2 * C], F32)
            nc.sync.dma_start(out=RABa, in_=RABd[0:C, :])
            RABx = sing.tile([3, 2 * C], F32)
            nc.sync.dma_start(out=RABx, in_=RABd[C:C + 3, :])
            RCNa = sing.tile([C, 2 * C], F32)
            nc.sync.dma_start(out=RCNa, in_=RCNd[0:C, :])
            RCNx = sing.tile([3, 2 * C], F32)
            nc.sync.dma_start(out=RCNx, in_=RCNd[C:C + 3, :])
            RCNb = sing.tile([1, 2 * C], F32)
            nc.sync.dma_start(out=RCNb, in_=RCNd[C + 3:C + 4, :])
            # negate the xyz rows: c_n = G1c - G1x + b1, d_n = bv - Gvx
            nc.vector.tensor_scalar_mul(RCNx, RCNx, -1.0)
            w2rep = sing.tile([128, H], F32)
            nc.sync.dma_start(out=w2rep, in_=w2d[:, :])
            gRow = sing.tile([1, 2 * C], F32)
            nc.sync.dma_start(out=gRow, in_=gbd[0:1, :])
            bRow = sing.tile([1, 2 * C], F32)
            nc.sync.dma_start(out=bRow, in_=gbd[1:2, :])
            RO = sing.tile([C + 1, C], F32)
            nc.sync.dma_start(out=RO, in_=ROd[:, :])
            gbo = sing.tile([C, 2], F32)
            nc.sync.dma_start(out=gbo, in_=gbod[:, :])
            F_sbq = sing.tile([C, NQ], F32)
            nc.sync.dma_start(out=F_sbq, in_=fc[:, 0:NQ])
            ones1 = sing.tile([1, 128], F32)
            nc.vector.memset(ones1, 1.0)
            ones128 = sing.tile([128, 1], F32)
            nc.vector.memset(ones128, 1.0)
            # identity for PE transpose
            identI = sing.tile([128, 128], mybir.dt.int32)
            nc.gpsimd.iota(identI, pattern=[[1, 128]], base=0, channel_multiplier=-1)
            ident = sing.tile([128, 128], F32)
            nc.vector.tensor_scalar(ident, identI, 0.0, scalar2=None, op0=ALU.is_equal)

            # sq/2 of candidate points -> row 3 of C4
            XTsq = lscp.tile([128, NA * 3], F32, tag="lsc")
            nc.scalar.activation(XTsq, XT.rearrange("p a d -> p (a d)"),
                                 ACTF.Square, scale=SQRT_HALF)
            SQ2 = sing.tile([128, NA], F32)
            nc.vector.tensor_reduce(out=SQ2, in_=XTsq.rearrange("p (a d) -> p a d", d=3),
                                    axis=AX.X, op=ALU.add)
            sqd = dram.tile([128, NA], F32)
            nc.sync.dma_start(out=sqd, in_=SQ2)
            nc.sync.dma_start(out=C4[3:4, :].rearrange("o (a p) -> o a p", p=128),
                              in_=sqd[:, :].rearrange("p a -> a p"))

            idxall = sing.tile([128, NT * K], U32)
            hv_d = dram.tile([NQ, K * 2 * C], F32)
            cnall = sing.tile([128, NT, 2 * C], F32)
            stat_ps = stps.tile([1, 2 * C], F32)    # sum(h|v)
            stat2_ps = st2ps.tile([1, 2 * C], F32)  # sumsq(h|v)
            # accumulated over all tiles and k-slices by per-k PE ones-matmuls
            # (two separate PSUM banks: one accumulation group per bank)

            NB2 = N // 1024   # nd psum tiles per query tile

            # ---------------- P1: kNN + BN stats ----------------
            # 1-tile software pipeline: nd(t+1) is emitted before the stats
            # matmuls of tile t so the in-order PE queue computes the next
            # tile's distances during this tile's topk instead of stalling
            # on the gather->add->square chain that feeds the stats.
            def emit_nd(t):
                qs = slice(t * 128, (t + 1) * 128)
                q4t = q4ts[t % 2]
                nc.scalar.copy(q4t[0:3, :], C4[0:3, qs])
                nd = ndp.tile([128, N], F32)
                for b2 in range(2 * NB2):
                    cs = slice(b2 * 512, (b2 + 1) * 512)
                    ps = ndps.tile([128, 512], F32)
                    nc.tensor.matmul(ps, lhsT=q4t, rhs=C4[:, cs],
                                     start=True, stop=True)
                    nc.scalar.copy(nd[:, cs], ps)
                return nd

            def emit_stats(t, G, sq3):
                for kk in range(K):
                    nc.tensor.matmul(stat_ps, lhsT=ones128,
                                     rhs=G[:, kk, :],
                                     start=(t == 0 and kk == 0),
                                     stop=(t == NT - 1 and kk == K - 1))
                    nc.tensor.matmul(stat2_ps, lhsT=ones128,
                                     rhs=sq3[:, kk, :],
                                     start=(t == 0 and kk == 0),
                                     stop=(t == NT - 1 and kk == K - 1))

            pend_stats = []
            pend_add = None
            nd = emit_nd(0)
            # point projection tables -> DRAM Qtab [N, 128]
            Qtab = dram.tile([N, 2 * C], F32)
            for a in range(NA):
                pt = slice(a * 128, (a + 1) * 128)
                fstr = shp.tile([C, 128], F32, tag="fstr")
                nc.sync.dma_start(out=fstr, in_=fc[:, pt])
                ps = cnps.tile([128, 2 * C], F32, tag="cps")
                nc.tensor.matmul(ps, lhsT=fstr, rhs=RABa,
                                 start=True, stop=False)
                nc.tensor.matmul(ps, lhsT=C4[0:3, pt], rhs=RABx,
                                 start=False, stop=True)
                tsb = cnp_sb.tile([128, 2 * C], F32)
                nc.scalar.copy(tsb, ps)
                nc.sync.dma_start(out=Qtab[pt, :], in_=tsb)

            for t in range(NT):
                qs = slice(t * 128, (t + 1) * 128)
                # exact top-16 (5 passes)
                v8a = smp.tile([128, 8], F32)
                v8b = smp.tile([128, 8], F32)
                nc.vector.max(out=v8a, in_=nd)
                nc.vector.max_index(out=idxall[:, t * K:t * K + 8], in_max=v8a, in_values=nd)
                # first half of the gathers can start as soon as the first
                # 8 indices are known, overlapping the rest of the topk
                G = gp.tile([128, K, 2 * C], F32, tag="g")
                for kk in range(8):
                    nc.gpsimd.indirect_dma_start(
                        out=G[:, kk, :], out_offset=None, in_=Qtab[:, :],
                        in_offset=IndirectOffsetOnAxis(
                            ap=idxall[:, t * K + kk:t * K + kk + 1], axis=0))
                nc.vector.match_replace(out=nd, in_to_replace=v8a, in_values=nd,
                                        imm_value=NEG)
                nc.vector.max(out=v8b, in_=nd)
                nc.vector.max_index(out=idxall[:, t * K + 8:t * K + 16], in_max=v8b,
                                    in_values=nd)
                # CN = [c_q | d_q] -> persistent SBUF stash
                cps = cnps.tile([128, 2 * C], F32)
                nc.tensor.matmul(cps, lhsT=F_sbq[:, qs], rhs=RCNa, start=True, stop=False)
                nc.tensor.matmul(cps, lhsT=C4[0:3, qs], rhs=RCNx, start=False, stop=False)
                nc.tensor.matmul(cps, lhsT=ones1, rhs=RCNb, start=False, stop=True)
                nc.scalar.copy(cnall[:, t, :], cps)
                # second half of the gathers (one indirect DMA per k: the
                # hardware DGE only supports one offset per partition)
                for kk in range(8, K):
                    nc.gpsimd.indirect_dma_start(
                        out=G[:, kk, :], out_offset=None, in_=Qtab[:, :],
                        in_offset=IndirectOffsetOnAxis(
                            ap=idxall[:, t * K + kk:t * K + kk + 1], axis=0))
                # previous tile's CN-add + square (one tile late so no
                # engine waits on its gather between two topk chains).  The
                # broadcast CN is materialized on ACT and accumulated into
                # the gathered rows by a SWDGE CCE-add DMA - no DVE work.
                if pend_add is not None:
                    tp, Gp = pend_add
                    if tp % 2 == 0:
                        # even tiles: CCE-add via SWDGE (Pool has the margin)
                        cnK = scrp.tile([128, K, 2 * C], F32, tag="hvn")
                        nc.scalar.activation(cnK, _b(cnall[:, tp, :], 1, K),
                                             ACTF.Copy)
                        nc.gpsimd.dma_start(
                            out=Gp.rearrange("p k c -> p (k c)"),
                            in_=cnK.rearrange("p k c -> p (k c)"),
                            accum_op=ALU.add)
                    else:
                        # odd tiles: DVE add (Pool is saturated by gathers)
                        nc.vector.scalar_tensor_tensor(
                            out=Gp, in0=Gp, scalar=0.0,
                            in1=_b(cnall[:, tp, :], 1, K),
                            op0=ALU.bypass, op1=ALU.add)
                    sqh = scrp.tile([128, K * 2 * C], F32, tag="hvn")
                    nc.scalar.activation(sqh, Gp.rearrange("p k c -> p (k c)"),
                                         ACTF.Square)
                    nc.sync.dma_start(out=hv_d[tp * 128:(tp + 1) * 128, :],
                                      in_=Gp.rearrange("p k c -> p (k c)"))
                    pend_stats.append((tp, Gp,
                                       sqh.rearrange("p (k c) -> p k c", k=K)))
                pend_add = (t, G)
                # next tile's distances (PE) ...
                if t + 1 < NT:
                    nd = emit_nd(t + 1)
                # ... then the pending stats: per-channel sum & sumsq over
                # (q, k) accumulated on the PE across all tiles and k-slices
                if pend_stats:
                    emit_stats(*pend_stats.pop(0))
            tp, Gp = pend_add
            cnK = scrp.tile([128, K, 2 * C], F32, tag="hvn")
            nc.scalar.activation(cnK, _b(cnall[:, tp, :], 1, K), ACTF.Copy)
            nc.gpsimd.dma_start(
                out=Gp.rearrange("p k c -> p (k c)"),
                in_=cnK.rearrange("p k c -> p (k c)"), accum_op=ALU.add)
            sqh = scrp.tile([128, K * 2 * C], F32, tag="hvn")
            nc.scalar.activation(sqh, Gp.rearrange("p k c -> p (k c)"), ACTF.Square)
            nc.sync.dma_start(out=hv_d[tp * 128:(tp + 1) * 128, :],
                              in_=Gp.rearrange("p k c -> p (k c)"))
            pend_stats.append((tp, Gp, sqh.rearrange("p (k c) -> p k c", k=K)))
            while pend_stats:
                emit_stats(*pend_stats.pop(0))

            def emit_gather(t):
                # broadcast-fill with the t/s shift on ACT, then the stashed
                # HV accumulates on top via a SWDGE CCE-add DMA (keeps the
                # per-tile broadcast add off the DVE critical path)
                G2 = gp.tile([128, K, 2 * C], F32, tag="g")
                nc.scalar.activation(G2, _b(t128[:, :], 1, K), ACTF.Copy)
                nc.gpsimd.dma_start(
                    out=G2.rearrange("p k c -> p (k c)"),
                    in_=hv_d[t * 128:(t + 1) * 128, :], accum_op=ALU.add)
                return G2

            def emit_a(t, G2=None):
                if G2 is None:
                    G2 = emit_gather(t)
                HVn = scrp.tile([128, K * 2 * C], F32, tag="hvn")
                nc.scalar.activation(HVn, G2.rearrange("p k c -> p (k c)"), ACTF.Relu)
                HVn3 = HVn.rearrange("p (k c) -> p k c", k=K)
                # logits & softmax over K (no max-subtraction: logits are
                # O(10) and exp is safe in f32)
                lsc = lscp.tile([128, K, H], F32, tag="lsc")
                nc.vector.scalar_tensor_tensor(
                    out=lsc, in0=HVn3[:, :, 0:C], scalar=0.0,
                    in1=_b(w2p[:, :], 1, K), op0=ALU.bypass, op1=ALU.mult)
                logit = smp.tile([128, K], F32)
                nc.vector.tensor_reduce(out=logit, in_=lsc, axis=AX.X, op=ALU.add)
                ex = smp.tile([128, K], F32)
                sume = smp.tile([128, 1], F32)
                nc.scalar.activation(ex, logit, ACTF.Exp, accum_out=sume)
                rec = smp.tile([128, 1], F32)
                nc.vector.reciprocal(rec, sume)
                return HVn3, ex, rec

            def emit_b(t, HVn3, ex, rec):
                qs = slice(t * 128, (t + 1) * 128)
                # weighted sum over K with unnormalized weights (in place
                # over the value half of HVn)
                prod = HVn3[:, :, C:2 * C]
                nc.vector.scalar_tensor_tensor(
                    out=prod, in0=prod, scalar=0.0,
                    in1=_b(ex[:, :], 2, C), op0=ALU.bypass, op1=ALU.mult)
                outq = smp.tile([128, C], F32, tag="outq")
                nc.vector.tensor_reduce(out=outq, in_=prod.rearrange("p k c -> p c k"),
                                        axis=AX.X, op=ALU.add)
                nc.vector.tensor_scalar_mul(outq, outq, rec[:, 0:1])
                # o = (out @ Wo.T * sv) + bo, via transpose + matmul
                tps = pops.tile([C, 128], F32, tag="po")
                nc.tensor.transpose(tps, outq, ident)
                ot5 = ot5s[t % 2]
                nc.scalar.copy(ot5[0:C, :], tps)
                ops_ = pops.tile([C, 128], F32, tag="po")
                nc.tensor.matmul(ops_, lhsT=ROp, rhs=ot5, start=True, stop=True)
                osb = otp_sb.tile([C, 128], F32, tag="osb")
                nc.scalar.activation(osb, ops_, ACTF.Copy,
                                     accum_out=osums[:, t:t + 1])
                nc.sync.dma_start(out=ostash_d[:, qs], in_=osb)
                osq = otp_sb.tile([C, 128], F32, tag="osq")
                nc.scalar.activation(osq, ops_, ACTF.Square,
                                     accum_out=osums2[:, t:t + 1])


            # ---------------- AR1 ----------------
            stats_sb = sing.tile([1, 4 * C], F32)
            nc.vector.tensor_copy(stats_sb[:, 0:2 * C], stat_ps)
            nc.vector.tensor_copy(stats_sb[:, 2 * C:4 * C], stat2_ps)
            bi1 = dram.tile([1, 4 * C], F32)
            bo1 = dram.tile([1, 4 * C], F32)
            nc.sync.dma_start(out=bi1, in_=stats_sb)
            if n_cores > 1:
                nc.gpsimd.collective_compute(
                    "AllReduce", ALU.add,
                    replica_groups=[list(range(n_cores))],
                    ins=[bi1[:, :].opt()], outs=[bo1[:, :].opt()])
            else:
                nc.sync.dma_start(out=bo1[:, :], in_=bi1[:, :])
            stats2 = sing.tile([1, 4 * C], F32)
            nc.sync.dma_start(out=stats2, in_=bo1)

            mean = sing.tile([1, 2 * C], F32)
            nc.vector.tensor_scalar_mul(mean, stats2[:, 0:2 * C], 1.0 / tot_pairs)
            var = sing.tile([1, 2 * C], F32)
            nc.vector.tensor_scalar_mul(var, stats2[:, 2 * C:4 * C], 1.0 / tot_pairs)
            msq = sing.tile([1, 2 * C], F32)
            nc.vector.tensor_mul(msq, mean, mean)
            nc.vector.tensor_sub(var, var, msq)
            nc.vector.tensor_scalar_add(var, var, EPS)
            sdv = sing.tile([1, 2 * C], F32)
            nc.scalar.sqrt(sdv, var)
            rstd = sing.tile([1, 2 * C], F32)
            nc.vector.reciprocal(rstd, sdv)
            svec = sing.tile([1, 2 * C], F32)
            nc.vector.tensor_mul(svec, gRow, rstd)
            tvec = sing.tile([1, 2 * C], F32)
            nc.vector.tensor_mul(tvec, mean, svec)
            nc.vector.tensor_sub(tvec, bRow, tvec)
            sinv = sing.tile([1, 2 * C], F32)
            nc.vector.reciprocal(sinv, svec)
            tps_row = sing.tile([1, 2 * C], F32)   # t/s row for CN'
            nc.vector.tensor_mul(tps_row, tvec, sinv)
            # replicate s_h across partitions via PE rank-1 broadcast
            # (0-stride partition DMA is not supported by the hardware DGE)
            srep_ps = cnps.tile([128, H], F32, tag="cps")
            nc.tensor.matmul(srep_ps, lhsT=ones1, rhs=svec[:, 0:C],
                             start=True, stop=True)
            srep = sing.tile([128, H], F32)
            nc.scalar.copy(srep, srep_ps)
            sdr = dram.tile([1, 2 * C], F32)
            nc.sync.dma_start(out=sdr, in_=svec)
            sv64 = sing.tile([C, 1], F32)
            nc.sync.dma_start(out=sv64, in_=sdr[0, C:2 * C].rearrange("(p o) -> p o", o=1))
            # fold s into w2 and Wo
            w2p = sing.tile([128, H], F32)
            nc.vector.tensor_mul(w2p, w2rep, srep)
            ROp = sing.tile([C + 1, C], F32)
            nc.vector.tensor_mul(ROp[0:C, :], RO[0:C, :], sv64.to_broadcast([C, C]))
            nc.vector.tensor_copy(ROp[C:C + 1, :], RO[C:C + 1, :])

            t128_ps = cnps.tile([128, 2 * C], F32, tag="cps")
            nc.tensor.matmul(t128_ps, lhsT=ones1, rhs=tps_row, start=True, stop=True)
            t128 = sing.tile([128, 2 * C], F32)
            nc.scalar.copy(t128, t128_ps)
            pre_g = [emit_gather(0), emit_gather(1)]
            # (the t/s shift is applied per-tile in P2 on top of the
            # stashed HV = gathered + CN)
            ostash_d = dram.tile([C, NQ], F32)
            osums = sing.tile([C, NT], F32)
            osums2 = sing.tile([C, NT], F32)

            # pre-initialized [.; ones] staging tiles for the output matmul
            ot5_0 = sing.tile([C + 1, 128], F32)
            ot5_1 = sing.tile([C + 1, 128], F32)
            ot5s = [ot5_0, ot5_1]
            nc.vector.memset(ot5s[0][C:C + 1, :], 1.0)
            nc.vector.memset(ot5s[1][C:C + 1, :], 1.0)

            # ---------------- P2: attention + value + output proj ----------------
            # 1-tile software pipeline: stage A(t) = gather + attention front
            # end; stage B(t-1) = value-weighting + output projection.  B is
            # emitted one tile late so the in-order Pool/PE queues never block
            # the next tile's gather on this tile's back end.
            pend = []
            for t in range(NT):
                pend.append((t, emit_a(t, pre_g.pop(0))))
                if t + 2 < NT:
                    pre_g.append(emit_gather(t + 2))
                if len(pend) > 2:
                    tb, ab = pend.pop(0)
                    emit_b(tb, *ab)
            while pend:
                tb, ab = pend.pop(0)
                emit_b(tb, *ab)

            # ---------------- AR2 ----------------
            ost = sing.tile([C, 2], F32)
            nc.vector.tensor_reduce(out=ost[:, 0:1], in_=osums, axis=AX.X, op=ALU.add)
            nc.vector.tensor_reduce(out=ost[:, 1:2], in_=osums2, axis=AX.X, op=ALU.add)
            bi2 = dram.tile([C, 2], F32)
            bo2 = dram.tile([C, 2], F32)
            nc.sync.dma_start(out=bi2, in_=ost)
            if n_cores > 1:
                nc.gpsimd.collective_compute(
                    "AllReduce", ALU.add,
                    replica_groups=[list(range(n_cores))],
                    ins=[bi2[:, :].opt()], outs=[bo2[:, :].opt()])
            else:
                nc.sync.dma_start(out=bo2[:, :], in_=bi2[:, :])
            ost2 = sing.tile([C, 2], F32)
            nc.sync.dma_start(out=ost2, in_=bo2)
            omean = sing.tile([C, 1], F32)
            nc.vector.tensor_scalar_mul(omean, ost2[:, 0:1], 1.0 / tot_pts)
            ovar = sing.tile([C, 1], F32)
            nc.vector.tensor_scalar_mul(ovar, ost2[:, 1:2], 1.0 / tot_pts)
            omsq = sing.tile([C, 1], F32)
            nc.vector.tensor_mul(omsq, omean, omean)
            nc.vector.tensor_sub(ovar, ovar, omsq)
            nc.vector.tensor_scalar_add(ovar, ovar, EPS)
            osd = sing.tile([C, 1], F32)
            nc.scalar.sqrt(osd, ovar)
            orst = sing.tile([C, 1], F32)
            nc.vector.reciprocal(orst, osd)
            so = sing.tile([C, 1], F32)
            nc.vector.tensor_mul(so, gbo[:, 0:1], orst)
            to = sing.tile([C, 1], F32)
            nc.vector.tensor_mul(to, omean, so)
            nc.vector.tensor_sub(to, gbo[:, 1:2], to)

            # ---------------- P3: BN + relu + residual ----------------
            P3CH = min(1024, NQ)
            for j in range(NQ // P3CH):
                js = slice(j * P3CH, (j + 1) * P3CH)
                ob = scrp.tile([C, P3CH], F32, tag="hvn")
                nc.sync.dma_start(out=ob, in_=ostash_d[:, js])
                nc.scalar.activation(ob, ob, ACTF.Relu, bias=to[:, 0:1],
                                     scale=so[:, 0:1])
                nc.vector.scalar_tensor_tensor(out=ob, in0=ob, scalar=0.0,
                                                in1=F_sbq[:, js],
                                                op0=ALU.bypass, op1=ALU.add)
                nc.sync.dma_start(out=outd[:, js], in_=ob)

    nc.compile()
    return nc


def make_in_maps(xyz, feats, W1, b1, g1, be1, W2, b2, Wv, bv, gv, bev,
                 Wo, bo, go, beo, n_cores=8, N=8192, NQ=4096):
    """Shard/lay out the full inputs into per-core input dicts (layout only)."""
    f32 = np.float32
    W1 = np.asarray(W1, f32)
    Wv = np.asarray(Wv, f32)
    # RAB: rows [in-ch(64); xyz(3)], cols [A(64) | Bv(64)]
    RAB = np.concatenate([
        np.concatenate([W1[:, C:2 * C].T, W1[:, 2 * C:2 * C + 3].T], axis=0),
        np.concatenate([Wv[:, 0:C].T, Wv[:, C:C + 3].T], axis=0),
    ], axis=1).astype(f32)
    RCN = np.concatenate([
        np.concatenate([W1[:, 0:C].T, W1[:, 2 * C:2 * C + 3].T,
                        np.asarray(b1, f32)[None, :]], axis=0),
        np.concatenate([np.zeros((C, C), f32), Wv[:, C:C + 3].T,
                        np.asarray(bv, f32)[None, :]], axis=0),
    ], axis=1).astype(f32)
    w2rep = np.ascontiguousarray(np.broadcast_to(np.asarray(W2, f32)[0], (128, H)))
    gbp = np.stack([np.concatenate([np.asarray(g1, f32), np.asarray(gv, f32)]),
                    np.concatenate([np.asarray(be1, f32), np.asarray(bev, f32)])])
    RO = np.concatenate([np.asarray(Wo, f32).T, np.asarray(bo, f32)[None, :]], axis=0)
    gbo = np.stack([np.asarray(go, f32), np.asarray(beo, f32)], axis=1)

    xyz = np.asarray(xyz, f32)
    feats = np.asarray(feats, f32)
    halves = n_cores // xyz.shape[0]      # cores per batch element
    in_maps = []
    for c in range(n_cores):
        b = c // halves
        h = c % halves
        xb = np.roll(xyz[b], -h * NQ, axis=1)
        fb = np.roll(feats[b], -h * NQ, axis=1)
        in_maps.append({
            "xyzc": np.ascontiguousarray(xb),
            "xyzT": np.ascontiguousarray(xb.T),
            "fc": np.ascontiguousarray(fb),
            "RAB": RAB, "RCN": RCN, "w2rep": w2rep, "gb": gbp,
            "RO": np.ascontiguousarray(RO), "gbo": np.ascontiguousarray(gbo),
        })
    return in_maps


_NC_CACHE = {}


def kernel(**inputs):
    from concourse.bass_utils import run_bass_kernel_spmd
    B, _, N = inputs["xyz"].shape
    n_cores = 8
    NQ = N * B // n_cores
    key = (N, NQ, n_cores)
    if key not in _NC_CACHE:
        _NC_CACHE[key] = build_nc(N=N, NQ=NQ, n_cores=n_cores)
    nc = _NC_CACHE[key]
    in_maps = make_in_maps(n_cores=n_cores, N=N, NQ=NQ, **inputs)
    res = run_bass_kernel_spmd(nc, in_maps, core_ids=list(range(n_cores)))
    halves = n_cores // B
    out = np.empty((B, C, N), np.float32)
    for c in range(n_cores):
        b, h = c // halves, c % halves
        out[b][:, h * NQ:(h + 1) * NQ] = res.results[c]["out"]
    return out


# revision 42
# speedup vs baseline: 1.0399x; 1.0014x over previous
"""Trainium2 Bass kernel for nn_AttentionTopologyModule (point-cloud kNN attention).

Contract: kernel(**inputs) takes the FULL unsharded inputs (as produced by
setup_inputs) and returns the FULL [B, C, N] output.  Internally the work is
sharded data-parallel over (batch, query-half): 8 cores, each handling 4096
query points of one batch element (candidates = all 8192 points of that batch
element).  The tiny MLP weights are replicated to every core.

Algorithm per core (all model arithmetic on device):
  setup:  load xyz/feats; sq_m = ||x_m||^2/2; point-projection table
          Qtab[j,:] = [A_j | Bv_j] in DRAM where
          A = feats@W1n.T + xyz@W1x.T  (attn branch, neighbor part)
          Bv = feats@Wvn.T + xyz@Wvx.T (value branch, neighbor part)
  P1 (per 128-query tile, 1-tile software pipeline so the in-order PE/DVE
      queues never stall between tiles):
      nd[q,m] = x_q . x_m - ||x_m||^2/2   (PE matmul, f32; row-equivalent
                ordering to -distance)  -> exact top-16 via DVE
                max8/max_index/match_replace/max8/max_index (the 5 full
                passes are the kernel's irreducible critical path)
      ONE batched indirect DMA gathers the K=16 Qtab rows per query,
      CN[q,:] = [c_q | d_q] per-query offsets via PE matmul (cached in
      SBUF), HV = gathered + CN (DVE, deferred one tile), per-channel
      sum/sumsq of HV accumulated on the PE via per-k ones-matmuls into
      two PSUM banks (one accumulation group per bank).
  AR1:  AllReduce the BN batch stats (training-mode BatchNorm over the whole
        batch spans all cores); fold scale s into w2 / Wo (relu is positively
        homogeneous, gamma=1>0), fold t/s into the cached per-query offsets.
  P2 (A/B software pipeline; gathers pre-issued two tiles ahead):
        re-gather (batched indirect DMA), add CN' (=CN + t/s), relu,
        logits = h.w2' (DVE mult + reduce), softmax over K=16 without
        max-subtraction (logits are O(10)), exp fused with its row-sum on
        ACT, value weighting in place with unnormalized weights, 1/sum
        applied to the reduced row, o = out@(Wo.T*sv)+bo (PE transpose +
        matmul), o stats via ACT accumulators, o stashed to DRAM.
  AR2:  AllReduce o stats; s_o/t_o.
  P3:   BN+relu on stashed o (ACT per-partition affine), residual add
        feats, DMA out [64, 4096].
"""

import sys

import numpy as np

sys.path.insert(0, "/opt/trn_rl_repo")

import concourse.bacc as bacc
import concourse.bass as bass
import concourse.mybir as mybir
import concourse.tile as tile
from concourse.bass import IndirectOffsetOnAxis

F32 = mybir.dt.float32
U32 = mybir.dt.uint32
ALU = mybir.AluOpType
ACTF = mybir.ActivationFunctionType
AX = mybir.AxisListType

C = 64      # channels
K = 16      # neighbors
H = 64      # hidden dim
EPS = 1e-5
NEG = -1.0e30
SQRT_HALF = 0.7071067811865476


def _b(ap, ins_at, count):
    """Insert a broadcast (step 0) dim into an AP at position ins_at."""
    pat = [list(p) for p in ap.ap]
    pat = pat[:ins_at] + [[0, count]] + pat[ins_at:]
    return bass.AP(tensor=ap.tensor, offset=ap.offset, ap=pat)


def build_nc(N=8192, NQ=4096, n_cores=8, tot_pairs=None, tot_pts=None):
    """Build the per-core Bass program (SPMD: same program, per-core inputs)."""
    NT = NQ // 128          # query tiles
    NA = N // 128           # point tiles (tables)
    if tot_pairs is None:
        tot_pairs = n_cores * NQ * K    # elements per channel in BN1/BNv stats
    if tot_pts is None:
        tot_pts = n_cores * NQ          # elements per channel in BNo stats

    nc = bacc.Bacc("TRN2", target_bir_lowering=False, debug=False,
                   num_devices=n_cores)

    xyzc = nc.dram_tensor("xyzc", [3, N], F32, kind="ExternalInput")
    xyzT = nc.dram_tensor("xyzT", [N, 3], F32, kind="ExternalInput")
    fc = nc.dram_tensor("fc", [C, N], F32, kind="ExternalInput")
    RABd = nc.dram_tensor("RAB", [C + 3, 2 * C], F32, kind="ExternalInput")
    RCNd = nc.dram_tensor("RCN", [C + 4, 2 * C], F32, kind="ExternalInput")
    w2d = nc.dram_tensor("w2rep", [128, H], F32, kind="ExternalInput")
    gbd = nc.dram_tensor("gb", [2, 2 * C], F32, kind="ExternalInput")
    ROd = nc.dram_tensor("RO", [C + 1, C], F32, kind="ExternalInput")
    gbod = nc.dram_tensor("gbo", [C, 2], F32, kind="ExternalInput")
    outd = nc.dram_tensor("out", [C, NQ], F32, kind="ExternalOutput")

    with tile.TileContext(nc) as tc:
        import contextlib
        ctx = contextlib.ExitStack()
        with ctx:
            sing = ctx.enter_context(tc.tile_pool(name="sing", bufs=1))
            dram = ctx.enter_context(tc.tile_pool(name="dram", bufs=1, space="DRAM"))
            ndp = ctx.enter_context(tc.tile_pool(name="ndp", bufs=2))
            gp = ctx.enter_context(tc.tile_pool(name="gp", bufs=3))
            scrp = ctx.enter_context(tc.tile_pool(name="scrp", bufs=3))
            lscp = ctx.enter_context(tc.tile_pool(name="lscp", bufs=2))
            cnp_sb = ctx.enter_context(tc.tile_pool(name="cnsb", bufs=1))
            shp = ctx.enter_context(tc.tile_pool(name="shp", bufs=2))
            smp = ctx.enter_context(tc.tile_pool(name="smp", bufs=3))
            otp_sb = ctx.enter_context(tc.tile_pool(name="otsb", bufs=2))
            # PSUM pools (8 banks total): nd 2x2 banks, cn 1, stats 1, tr 1, oT 1
            ndps = ctx.enter_context(tc.tile_pool(name="ndps", bufs=2, space="PSUM"))
            cnps = ctx.enter_context(tc.tile_pool(name="cnps", bufs=1, space="PSUM"))
            stps = ctx.enter_context(tc.tile_pool(name="stps", bufs=1, space="PSUM"))
            st2ps = ctx.enter_context(tc.tile_pool(name="st2ps", bufs=1, space="PSUM"))
            pops = ctx.enter_context(tc.tile_pool(name="pops", bufs=2, space="PSUM"))

            # ---------------- setup ----------------
            # xyz loads first: they gate the sq-row chain and tile 0's nd
            C4 = sing.tile([4, N], F32)
            nc.sync.dma_start(out=C4[0:3, :], in_=xyzc[:, :])
            q4t_0 = sing.tile([4, 128], F32)
            q4t_1 = sing.tile([4, 128], F32)
            nc.vector.memset(q4t_0, -1.0)
            nc.vector.memset(q4t_1, -1.0)
            q4ts = [q4t_0, q4t_1]
            XT = scrp.tile([128, NA, 3], F32, tag="hvn")
            nc.sync.dma_start(out=XT, in_=xyzT[:, :].rearrange("(a p) d -> p a d", p=128))
            RABa = sing.tile([C, 2 * C], F32)
            nc.sync.dma_start(out=RABa, in_=RABd[0:C, :])
            RABx = sing.tile([3, 2 * C], F32)
            nc.sync.dma_start(out=RABx, in_=RABd[C:C + 3, :])
            RCNa = sing.tile([C, 2 * C], F32)
            nc.sync.dma_start(out=RCNa, in_=RCNd[0:C, :])
            RCNx = sing.tile([3, 2 * C], F32)
            nc.sync.dma_start(out=RCNx, in_=RCNd[C:C + 3, :])
            RCNb = sing.tile([1, 2 * C], F32)
            nc.sync.dma_start(out=RCNb, in_=RCNd[C + 3:C + 4, :])
            # negate the xyz rows: c_n = G1c - G1x + b1, d_n = bv - Gvx
            nc.vector.tensor_scalar_mul(RCNx, RCNx, -1.0)
            w2rep = sing.tile([128, H], F32)
            nc.sync.dma_start(out=w2rep, in_=w2d[:, :])
            gRow = sing.tile([1, 2 * C], F32)
            nc.sync.dma_start(out=gRow, in_=gbd[0:1, :])
            bRow = sing.tile([1, 2 * C], F32)
            nc.sync.dma_start(out=bRow, in_=gbd[1:2, :])
            RO = sing.tile([C + 1, C], F32)
            nc.sync.dma_start(out=RO, in_=ROd[:, :])
            gbo = sing.tile([C, 2], F32)
            nc.sync.dma_start(out=gbo, in_=gbod[:, :])
            F_sbq = sing.tile([C, NQ], F32)
            nc.sync.dma_start(out=F_sbq, in_=fc[:, 0:NQ])
            ones1 = sing.tile([1, 128], F32)
            nc.vector.memset(ones1, 1.0)
            ones128 = sing.tile([128, 1], F32)
            nc.vector.memset(ones128, 1.0)
            # identity for PE transpose
            identI = sing.tile([128, 128], mybir.dt.int32)
            nc.gpsimd.iota(identI, pattern=[[1, 128]], base=0, channel_multiplier=-1)
            ident = sing.tile([128, 128], F32)
            nc.vector.tensor_scalar(ident, identI, 0.0, scalar2=None, op0=ALU.is_equal)

            # sq/2 of candidate points -> row 3 of C4
            XTsq = lscp.tile([128, NA * 3], F32, tag="lsc")
            nc.scalar.activation(XTsq, XT.rearrange("p a d -> p (a d)"),
                                 ACTF.Square, scale=SQRT_HALF)
            SQ2 = sing.tile([128, NA], F32)
            nc.vector.tensor_reduce(out=SQ2, in_=XTsq.rearrange("p (a d) -> p a d", d=3),
                                    axis=AX.X, op=ALU.add)
            sqd = dram.tile([128, NA], F32)
            nc.sync.dma_start(out=sqd, in_=SQ2)
            nc.sync.dma_start(out=C4[3:4, :].rearrange("o (a p) -> o a p", p=128),
                              in_=sqd[:, :].rearrange("p a -> a p"))

            idxall = sing.tile([128, NT * K], U32)
            hv_d = dram.tile([NQ, K * 2 * C], F32)
            cnall = sing.tile([128, NT, 2 * C], F32)
            stat_ps = stps.tile([1, 2 * C], F32)    # sum(h|v)
            stat2_ps = st2ps.tile([1, 2 * C], F32)  # sumsq(h|v)
            # accumulated over all tiles and k-slices by per-k PE ones-matmuls
            # (two separate PSUM banks: one accumulation group per bank)

            NB2 = N // 1024   # nd psum tiles per query tile

            # ---------------- P1: kNN + BN stats ----------------
            # 1-tile software pipeline: nd(t+1) is emitted before the stats
            # matmuls of tile t so the in-order PE queue computes the next
            # tile's distances during this tile's topk instead of stalling
            # on the gather->add->square chain that feeds the stats.
            def emit_nd(t):
                qs = slice(t * 128, (t + 1) * 128)
                q4t = q4ts[t % 2]
                nc.scalar.copy(q4t[0:3, :], C4[0:3, qs])
                nd = ndp.tile([128, N], F32)
                for b2 in range(2 * NB2):
                    cs = slice(b2 * 512, (b2 + 1) * 512)
                    ps = ndps.tile([128, 512], F32)
                    nc.tensor.matmul(ps, lhsT=q4t, rhs=C4[:, cs],
                                     start=True, stop=True)
                    nc.scalar.copy(nd[:, cs], ps)
                return nd

            def emit_stats(t, G, sq3):
                for kk in range(K):
                    nc.tensor.matmul(stat_ps, lhsT=ones128,
                                     rhs=G[:, kk, :],
                                     start=(t == 0 and kk == 0),
                                     stop=(t == NT - 1 and kk == K - 1))
                    nc.tensor.matmul(stat2_ps, lhsT=ones128,
                                     rhs=sq3[:, kk, :],
                                     start=(t == 0 and kk == 0),
                                     stop=(t == NT - 1 and kk == K - 1))

            pend_stats = []
            pend_add = None
            nd = emit_nd(0)
            # point projection tables -> DRAM Qtab [N, 128]
            Qtab = dram.tile([N, 2 * C], F32)
            for a in range(NA):
                pt = slice(a * 128, (a + 1) * 128)
                fstr = shp.tile([C, 128], F32, tag="fstr")
                nc.sync.dma_start(out=fstr, in_=fc[:, pt])
                ps = cnps.tile([128, 2 * C], F32, tag="cps")
                nc.tensor.matmul(ps, lhsT=fstr, rhs=RABa,
                                 start=True, stop=False)
                nc.tensor.matmul(ps, lhsT=C4[0:3, pt], rhs=RABx,
                                 start=False, stop=True)
                tsb = cnp_sb.tile([128, 2 * C], F32)
                nc.scalar.copy(tsb, ps)
                nc.sync.dma_start(out=Qtab[pt, :], in_=tsb)

            for t in range(NT):
                qs = slice(t * 128, (t + 1) * 128)
                # exact top-16 (5 passes)
                v8a = smp.tile([128, 8], F32)
                v8b = smp.tile([128, 8], F32)
                nc.vector.max(out=v8a, in_=nd)
                nc.vector.max_index(out=idxall[:, t * K:t * K + 8], in_max=v8a, in_values=nd)
                # first half of the gathers can start as soon as the first
                # 8 indices are known, overlapping the rest of the topk
                G = gp.tile([128, K, 2 * C], F32, tag="g")
                for kk in range(8):
                    nc.gpsimd.indirect_dma_start(
                        out=G[:, kk, :], out_offset=None, in_=Qtab[:, :],
                        in_offset=IndirectOffsetOnAxis(
                            ap=idxall[:, t * K + kk:t * K + kk + 1], axis=0))
                nc.vector.match_replace(out=nd, in_to_replace=v8a, in_values=nd,
                                        imm_value=NEG)
                nc.vector.max(out=v8b, in_=nd)
                nc.vector.max_index(out=idxall[:, t * K + 8:t * K + 16], in_max=v8b,
                                    in_values=nd)
                # CN = [c_q | d_q] -> persistent SBUF stash
                cps = cnps.tile([128, 2 * C], F32)
                nc.tensor.matmul(cps, lhsT=F_sbq[:, qs], rhs=RCNa, start=True, stop=False)
                nc.tensor.matmul(cps, lhsT=C4[0:3, qs], rhs=RCNx, start=False, stop=False)
                nc.tensor.matmul(cps, lhsT=ones1, rhs=RCNb, start=False, stop=True)
                nc.scalar.copy(cnall[:, t, :], cps)
                # second half of the gathers (one indirect DMA per k: the
                # hardware DGE only supports one offset per partition)
                for kk in range(8, K):
                    nc.gpsimd.indirect_dma_start(
                        out=G[:, kk, :], out_offset=None, in_=Qtab[:, :],
                        in_offset=IndirectOffsetOnAxis(
                            ap=idxall[:, t * K + kk:t * K + kk + 1], axis=0))
                # previous tile's CN-add + square (one tile late so no
                # engine waits on its gather between two topk chains).  The
                # broadcast CN is materialized on ACT and accumulated into
                # the gathered rows by a SWDGE CCE-add DMA - no DVE work.
                if pend_add is not None:
                    tp, Gp = pend_add
                    if tp % 2 == 0:
                        # even tiles: CCE-add via SWDGE (Pool has the margin)
                        cnK = scrp.tile([128, K, 2 * C], F32, tag="hvn")
                        nc.scalar.activation(cnK, _b(cnall[:, tp, :], 1, K),
                                             ACTF.Copy)
                        nc.gpsimd.dma_start(
                            out=Gp.rearrange("p k c -> p (k c)"),
                            in_=cnK.rearrange("p k c -> p (k c)"),
                            accum_op=ALU.add)
                    else:
                        # odd tiles: DVE add (Pool is saturated by gathers)
                        nc.vector.scalar_tensor_tensor(
                            out=Gp, in0=Gp, scalar=0.0,
                            in1=_b(cnall[:, tp, :], 1, K),
                            op0=ALU.bypass, op1=ALU.add)
                    sqh = scrp.tile([128, K * 2 * C], F32, tag="hvn")
                    nc.scalar.activation(sqh, Gp.rearrange("p k c -> p (k c)"),
                                         ACTF.Square)
                    nc.sync.dma_start(out=hv_d[tp * 128:(tp + 1) * 128, :],
                                      in_=Gp.rearrange("p k c -> p (k c)"))
                    pend_stats.append((tp, Gp,
                                       sqh.rearrange("p (k c) -> p k c", k=K)))
                pend_add = (t, G)
                # next tile's distances (PE) ...
                if t + 1 < NT:
                    nd = emit_nd(t + 1)
                # ... then the pending stats: per-channel sum & sumsq over
                # (q, k) accumulated on the PE across all tiles and k-slices
                if pend_stats:
                    emit_stats(*pend_stats.pop(0))
            tp, Gp = pend_add
            cnK = scrp.tile([128, K, 2 * C], F32, tag="hvn")
            nc.scalar.activation(cnK, _b(cnall[:, tp, :], 1, K), ACTF.Copy)
            nc.gpsimd.dma_start(
                out=Gp.rearrange("p k c -> p (k c)"),
                in_=cnK.rearrange("p k c -> p (k c)"), accum_op=ALU.add)
            sqh = scrp.tile([128, K * 2 * C], F32, tag="hvn")
            nc.scalar.activation(sqh, Gp.rearrange("p k c -> p (k c)"), ACTF.Square)
            nc.sync.dma_start(out=hv_d[tp * 128:(tp + 1) * 128, :],
                              in_=Gp.rearrange("p k c -> p (k c)"))
            pend_stats.append((tp, Gp, sqh.rearrange("p (k c) -> p k c", k=K)))
            while pend_stats:
                emit_stats(*pend_stats.pop(0))

            def emit_gather(t):
                # broadcast-fill with the t/s shift on ACT, then the stashed
                # HV accumulates on top via a SWDGE CCE-add DMA (keeps the
                # per-tile broadcast add off the DVE critical path)
                G2 = gp.tile([128, K, 2 * C], F32, tag="g")
                nc.scalar.activation(G2, _b(t128[:, :], 1, K), ACTF.Copy)
                nc.gpsimd.dma_start(
                    out=G2.rearrange("p k c -> p (k c)"),
                    in_=hv_d[t * 128:(t + 1) * 128, :], accum_op=ALU.add)
                return G2

            def emit_a(t, G2=None):
                if G2 is None:
                    G2 = emit_gather(t)
                HVn = scrp.tile([128, K * 2 * C], F32, tag="hvn")
                nc.scalar.activation(HVn, G2.rearrange("p k c -> p (k c)"), ACTF.Relu)
                HVn3 = HVn.rearrange("p (k c) -> p k c", k=K)
                # logits & softmax over K (no max-subtraction: logits are
                # O(10) and exp is safe in f32)
                lsc = lscp.tile([128, K, H], F32, tag="lsc")
                nc.vector.scalar_tensor_tensor(
                    out=lsc, in0=HVn3[:, :, 0:C], scalar=0.0,
                    in1=_b(w2p[:, :], 1, K), op0=ALU.bypass, op1=ALU.mult)
                logit = smp.tile([128, K], F32)
                nc.vector.tensor_reduce(out=logit, in_=lsc, axis=AX.X, op=ALU.add)
                ex = smp.tile([128, K], F32)
                sume = smp.tile([128, 1], F32)
                nc.scalar.activation(ex, logit, ACTF.Exp, accum_out=sume)
                rec = smp.tile([128, 1], F32)
                nc.vector.reciprocal(rec, sume)
                return HVn3, ex, rec

            def emit_b(t, HVn3, ex, rec):
                qs = slice(t * 128, (t + 1) * 128)
                # weighted sum over K with unnormalized weights (in place
                # over the value half of HVn)
                prod = HVn3[:, :, C:2 * C]
                nc.vector.scalar_tensor_tensor(
                    out=prod, in0=prod, scalar=0.0,
                    in1=_b(ex[:, :], 2, C), op0=ALU.bypass, op1=ALU.mult)
                outq = smp.tile([128, C], F32, tag="outq")
                nc.vector.tensor_reduce(out=outq, in_=prod.rearrange("p k c -> p c k"),
                                        axis=AX.X, op=ALU.add)
                nc.vector.tensor_scalar_mul(outq, outq, rec[:, 0:1])
                # o = (out @ Wo.T * sv) + bo, via transpose + matmul
                tps = pops.tile([C, 128], F32, tag="po")
                nc.tensor.transpose(tps, outq, ident)
                ot5 = ot5s[t % 2]
                nc.scalar.copy(ot5[0:C, :], tps)
                ops_ = pops.tile([C, 128], F32, tag="po")
                nc.tensor.matmul(ops_, lhsT=ROp, rhs=ot5, start=True, stop=True)
                osb = otp_sb.tile([C, 128], F32, tag="osb")
                nc.scalar.activation(osb, ops_, ACTF.Copy,
                                     accum_out=osums[:, t:t + 1])
                nc.sync.dma_start(out=ostash_d[:, qs], in_=osb)
                osq = otp_sb.tile([C, 128], F32, tag="osq")
                nc.scalar.activation(osq, ops_, ACTF.Square,
                                     accum_out=osums2[:, t:t + 1])


            # ---------------- AR1 ----------------
            stats_sb = sing.tile([1, 4 * C], F32)
            nc.vector.tensor_copy(stats_sb[:, 0:2 * C], stat_ps)
            nc.vector.tensor_copy(stats_sb[:, 2 * C:4 * C], stat2_ps)
            bi1 = dram.tile([1, 4 * C], F32)
            bo1 = dram.tile([1, 4 * C], F32)
            nc.sync.dma_start(out=bi1, in_=stats_sb)
            if n_cores > 1:
                nc.gpsimd.collective_compute(
                    "AllReduce", ALU.add,
                    replica_groups=[list(range(n_cores))],
                    ins=[bi1[:, :].opt()], outs=[bo1[:, :].opt()])
            else:
                nc.sync.dma_start(out=bo1[:, :], in_=bi1[:, :])
            stats2 = sing.tile([1, 4 * C], F32)
            nc.sync.dma_start(out=stats2, in_=bo1)

            mean = sing.tile([1, 2 * C], F32)
            nc.vector.tensor_scalar_mul(mean, stats2[:, 0:2 * C], 1.0 / tot_pairs)
            var = sing.tile([1, 2 * C], F32)
            nc.vector.tensor_scalar_mul(var, stats2[:, 2 * C:4 * C], 1.0 / tot_pairs)
            msq = sing.tile([1, 2 * C], F32)
            nc.vector.tensor_mul(msq, mean, mean)
            nc.vector.tensor_sub(var, var, msq)
            nc.vector.tensor_scalar_add(var, var, EPS)
            sdv = sing.tile([1, 2 * C], F32)
            nc.scalar.sqrt(sdv, var)
            rstd = sing.tile([1, 2 * C], F32)
            nc.vector.reciprocal(rstd, sdv)
            svec = sing.tile([1, 2 * C], F32)
            nc.vector.tensor_mul(svec, gRow, rstd)
            tvec = sing.tile([1, 2 * C], F32)
            nc.vector.tensor_mul(tvec, mean, svec)
            nc.vector.tensor_sub(tvec, bRow, tvec)
            sinv = sing.tile([1, 2 * C], F32)
            nc.vector.reciprocal(sinv, svec)
            tps_row = sing.tile([1, 2 * C], F32)   # t/s row for CN'
            nc.vector.tensor_mul(tps_row, tvec, sinv)
            # replicate s_h across partitions via PE rank-1 broadcast
            # (0-stride partition DMA is not supported by the hardware DGE)
            srep_ps = cnps.tile([128, H], F32, tag="cps")
            nc.tensor.matmul(srep_ps, lhsT=ones1, rhs=svec[:, 0:C],
                             start=True, stop=True)
            srep = sing.tile([128, H], F32)
            nc.scalar.copy(srep, srep_ps)
            sdr = dram.tile([1, 2 * C], F32)
            nc.sync.dma_start(out=sdr, in_=svec)
            sv64 = sing.tile([C, 1], F32)
            nc.sync.dma_start(out=sv64, in_=sdr[0, C:2 * C].rearrange("(p o) -> p o", o=1))
            # fold s into w2 and Wo
            w2p = sing.tile([128, H], F32)
            nc.vector.tensor_mul(w2p, w2rep, srep)
            ROp = sing.tile([C + 1, C], F32)
            nc.vector.tensor_mul(ROp[0:C, :], RO[0:C, :], sv64.to_broadcast([C, C]))
            nc.vector.tensor_copy(ROp[C:C + 1, :], RO[C:C + 1, :])

            t128_ps = cnps.tile([128, 2 * C], F32, tag="cps")
            nc.tensor.matmul(t128_ps, lhsT=ones1, rhs=tps_row, start=True, stop=True)
            t128 = sing.tile([128, 2 * C], F32)
            nc.scalar.copy(t128, t128_ps)
            pre_g = [emit_gather(0), emit_gather(1)]
            # (the t/s shift is applied per-tile in P2 on top of the
            # stashed HV = gathered + CN)
            ostash_d = dram.tile([C, NQ], F32)
            osums = sing.tile([C, NT], F32)
            osums2 = sing.tile([C, NT], F32)

            # pre-initialized [.; ones] staging tiles for the output matmul
            ot5_0 = sing.tile([C + 1, 128], F32)
            ot5_1 = sing.tile([C + 1, 128], F32)
            ot5s = [ot5_0, ot5_1]
            nc.vector.memset(ot5s[0][C:C + 1, :], 1.0)
            nc.vector.memset(ot5s[1][C:C + 1, :], 1.0)

            # ---------------- P2: attention + value + output proj ----------------
            # 1-tile software pipeline: stage A(t) = gather + attention front
            # end; stage B(t-1) = value-weighting + output projection.  B is
            # emitted one tile late so the in-order Pool/PE queues never block
            # the next tile's gather on this tile's back end.
            pend = []
            for t in range(NT):
                pend.append((t, emit_a(t, pre_g.pop(0))))
                if t + 2 < NT:
                    pre_g.append(emit_gather(t + 2))
                if len(pend) > 2:
                    tb, ab = pend.pop(0)
                    emit_b(tb, *ab)
            while pend:
                tb, ab = pend.pop(0)
                emit_b(tb, *ab)

            # ---------------- AR2 ----------------
            ost = sing.tile([C, 2], F32)
            nc.vector.tensor_reduce(out=ost[:, 0:1], in_=osums, axis=AX.X, op=ALU.add)
            nc.vector.tensor_reduce(out=ost[:, 1:2], in_=osums2, axis=AX.X, op=ALU.add)
            bi2 = dram.tile([C, 2], F32)
            bo2 = dram.tile([C, 2], F32)
            nc.sync.dma_start(out=bi2, in_=ost)
            if n_cores > 1:
                nc.gpsimd.collective_compute(
                    "AllReduce", ALU.add,
                    replica_groups=[list(range(n_cores))],
                    ins=[bi2[:, :].opt()], outs=[bo2[:, :].opt()])
            else:
                nc.sync.dma_start(out=bo2[:, :], in_=bi2[:, :])
            ost2 = sing.tile([C, 2], F32)
            nc.sync.dma_start(out=ost2, in_=bo2)
            omean = sing.tile([C, 1], F32)
            nc.vector.tensor_scalar_mul(omean, ost2[:, 0:1], 1.0 / tot_pts)
            ovar = sing.tile([C, 1], F32)
            nc.vector.tensor_scalar_mul(ovar, ost2[:, 1:2], 1.0 / tot_pts)
            omsq = sing.tile([C, 1], F32)
            nc.vector.tensor_mul(omsq, omean, omean)
            nc.vector.tensor_sub(ovar, ovar, omsq)
            nc.vector.tensor_scalar_add(ovar, ovar, EPS)
            osd = sing.tile([C, 1], F32)
            nc.scalar.sqrt(osd, ovar)
            orst = sing.tile([C, 1], F32)
            nc.vector.reciprocal(orst, osd)
            so = sing.tile([C, 1], F32)
            nc.vector.tensor_mul(so, gbo[:, 0:1], orst)
            to = sing.tile([C, 1], F32)
            nc.vector.tensor_mul(to, omean, so)
            nc.vector.tensor_sub(to, gbo[:, 1:2], to)

            # ---------------- P3: BN + relu + residual ----------------
            P3CH = min(2048, NQ)
            for j in range(NQ // P3CH):
                js = slice(j * P3CH, (j + 1) * P3CH)
                ob = scrp.tile([C, P3CH], F32, tag="hvn")
                nc.sync.dma_start(out=ob, in_=ostash_d[:, js])
                nc.scalar.activation(ob, ob, ACTF.Relu, bias=to[:, 0:1],
                                     scale=so[:, 0:1])
                nc.vector.scalar_tensor_tensor(out=ob, in0=ob, scalar=0.0,
                                                in1=F_sbq[:, js],
                                                op0=ALU.bypass, op1=ALU.add)
                nc.sync.dma_start(out=outd[:, js], in_=ob)

    nc.compile()
    return nc


def make_in_maps(xyz, feats, W1, b1, g1, be1, W2, b2, Wv, bv, gv, bev,
                 Wo, bo, go, beo, n_cores=8, N=8192, NQ=4096):
    """Shard/lay out the full inputs into per-core input dicts (layout only)."""
    f32 = np.float32
    W1 = np.asarray(W1, f32)
    Wv = np.asarray(Wv, f32)
    # RAB: rows [in-ch(64); xyz(3)], cols [A(64) | Bv(64)]
    RAB = np.concatenate([
        np.concatenate([W1[:, C:2 * C].T, W1[:, 2 * C:2 * C + 3].T], axis=0),
        np.concatenate([Wv[:, 0:C].T, Wv[:, C:C + 3].T], axis=0),
    ], axis=1).astype(f32)
    RCN = np.concatenate([
        np.concatenate([W1[:, 0:C].T, W1[:, 2 * C:2 * C + 3].T,
                        np.asarray(b1, f32)[None, :]], axis=0),
        np.concatenate([np.zeros((C, C), f32), Wv[:, C:C + 3].T,
                        np.asarray(bv, f32)[None, :]], axis=0),
    ], axis=1).astype(f32)
    w2rep = np.ascontiguousarray(np.broadcast_to(np.asarray(W2, f32)[0], (128, H)))
    gbp = np.stack([np.concatenate([np.asarray(g1, f32), np.asarray(gv, f32)]),
                    np.concatenate([np.asarray(be1, f32), np.asarray(bev, f32)])])
    RO = np.concatenate([np.asarray(Wo, f32).T, np.asarray(bo, f32)[None, :]], axis=0)
    gbo = np.stack([np.asarray(go, f32), np.asarray(beo, f32)], axis=1)

    xyz = np.asarray(xyz, f32)
    feats = np.asarray(feats, f32)
    halves = n_cores // xyz.shape[0]      # cores per batch element
    in_maps = []
    for c in range(n_cores):
        b = c // halves
        h = c % halves
        xb = np.roll(xyz[b], -h * NQ, axis=1)
        fb = np.roll(feats[b], -h * NQ, axis=1)
        in_maps.append({
            "xyzc": np.ascontiguousarray(xb),
            "xyzT": np.ascontiguousarray(xb.T),
            "fc": np.ascontiguousarray(fb),
            "RAB": RAB, "RCN": RCN, "w2rep": w2rep, "gb": gbp,
            "RO": np.ascontiguousarray(RO), "gbo": np.ascontiguousarray(gbo),
        })
    return in_maps


_NC_CACHE = {}


def kernel(**inputs):
    from concourse.bass_utils import run_bass_kernel_spmd
    B, _, N = inputs["xyz"].shape
    n_cores = 8
    NQ = N * B // n_cores
    key = (N, NQ, n_cores)
    if key not in _NC_CACHE:
        _NC_CACHE[key] = build_nc(N=N, NQ=NQ, n_cores=n_cores)
    nc = _NC_CACHE[key]
    in_maps = make_in_maps(n_cores=n_cores, N=N, NQ=NQ, **inputs)
    res = run_bass_kernel_spmd(nc, in_maps, core_ids=list(range(n_cores)))
    halves = n_cores // B
    out = np.empty((B, C, N), np.float32)
    for c in range(n_cores):
        b, h = c // halves, c % halves
        out[b][:, h * NQ:(h + 1) * NQ] = res.results[c]["out"]
    return out
